# revision 27
# baseline (speedup 1.0000x reference)
"""Trainium2 Bass kernel for nn_BertSelfAttention_7962869367489.

Dual-branch (self + cross/"knowledge") BERT attention, B=4, S=1024, K=512,
H=1024, NH=16, HD=64, fp32.

Sharding: 8 cores = (batch b in 0..3) x (head-group hg in 0..1, 8 heads each).
All six projections are column-split by head-group; per-head attention is
entirely core-local; output columns are disjoint per core, so the gather is a
pure concatenation (no collectives).

Per-core pipeline (matmul operands in f32r = TF32-like single-pass PE mode,
~1.5e-4 rel err, full PE rate at free-dim >= 256):
  1. PE-transpose hs -> hsT [H, S] woven with the V projection; ehs -> ehsT
     transposed inside the first attention fill slot.
  2. Projections: QT/KT/KQT = W.T @ hsT (+bias), KKT = Wkk.T @ ehsT
     (transposed outputs); Vaug/KVaug = hs @ Wv in normal orientation with an
     augmented column of 2.0 (the ctxT matmul then also produces
     2*softmax-denominator, folding the (ctx+kctx)*0.5 average into the
     normalization for free).
  3. Per head h: scoresT[t,s] = K_h @ Q_h^T via lhsT=KT_h, rhs=QT_h
     (contraction HD=64); exp on ACT with per-partition mask bias and 1/8
     scale; ctxT_aug[65,S] += Vaug_h^T @ E accumulated over key chunks;
     PE-transpose back to [S, 64|den]; normalize + merge both branches on DVE.
  Remaining projections are interleaved between attention branches so the
  PE-heavy projection work fills the ACT-bound (exp) gaps; knowledge branches
  run delayed by 2 heads; output is DMA'd in two head-halves.
"""
import numpy as np
from contextlib import ExitStack

import concourse.bacc as bacc
import concourse.tile as tile
import concourse.mybir as mybir
from concourse.bass_utils import run_bass_kernel_spmd

F32 = mybir.dt.float32
F32R = mybir.dt.float32r
AF = mybir.ActivationFunctionType
ALU = mybir.AluOpType

P = 128
S = 1024        # query length
TKS = 1024      # self-branch key length
TKK = 512       # knowledge-branch key length
H = 1024        # model dim (projection contraction)
HG = 512        # per-core output width (8 heads x 64)
NHL = 8         # heads per core
HD = 64
HC = H // P     # 8 contraction chunks
INV = 0.125     # 1/sqrt(64)

_CACHE = {}
_DUMP = False


def _build():
    nc = bacc.Bacc(target_bir_lowering=False, debug=False)

    hs = nc.dram_tensor("hs", [S, H], F32, kind="ExternalInput")
    ehs = nc.dram_tensor("ehs", [TKK, H], F32, kind="ExternalInput")
    w_in = {}
    b_in = {}
    for nm in ["q", "k", "v", "kq", "kk", "kv"]:
        w_in[nm] = nc.dram_tensor(f"w{nm}", [H, HG], F32R, kind="ExternalInput")
        b_in[nm] = nc.dram_tensor(f"b{nm}", [HG], F32, kind="ExternalInput")
    mask = nc.dram_tensor("mask", [TKS], F32, kind="ExternalInput")
    emask = nc.dram_tensor("emask", [TKK], F32, kind="ExternalInput")
    out = nc.dram_tensor("out", [S, HG], F32, kind="ExternalOutput")

    with tile.TileContext(nc) as tc, ExitStack() as ctx:
        const = ctx.enter_context(tc.tile_pool(name="const", bufs=1))
        persist = ctx.enter_context(tc.tile_pool(name="persist", bufs=1))
        stage = ctx.enter_context(tc.tile_pool(name="stage", bufs=1))
        wpool = ctx.enter_context(tc.tile_pool(name="wpool", bufs=2))
        wvpool = ctx.enter_context(tc.tile_pool(name="wvpool", bufs=8))
        attp = ctx.enter_context(tc.tile_pool(name="att", bufs=3))
        ctxp = ctx.enter_context(tc.tile_pool(name="ctxp", bufs=2))
        psproj = ctx.enter_context(tc.tile_pool(name="psproj", bufs=2, space="PSUM"))

        # ---- constants ----
        ident_dram = nc.inline_tensor(np.eye(P, dtype=np.float32), name="ident_c")
        ident = const.tile([P, P], F32)
        nc.sync.dma_start(out=ident, in_=ident_dram.ap())
        mask_sb = const.tile([P, TKS // P], F32)
        nc.gpsimd.dma_start(out=mask_sb, in_=mask.ap().rearrange("(kt p) -> p kt", p=P))
        emask_sb = const.tile([P, TKK // P], F32)
        nc.gpsimd.dma_start(out=emask_sb, in_=emask.ap().rearrange("(kt p) -> p kt", p=P))
        bias_col = {}
        for nm in ["q", "k", "kq", "kk"]:
            t = const.tile([P, 4], F32, name=f"bias_{nm}")
            nc.gpsimd.dma_start(out=t, in_=b_in[nm].ap().rearrange("(jt p) -> p jt", p=P))
            bias_col[nm] = t
        bias_row = {}
        for nm in ["v", "kv"]:
            t = const.tile([P, HG], F32, name=f"brow_{nm}")
            nc.gpsimd.dma_start(out=t, in_=b_in[nm].ap().unsqueeze(0).broadcast_to([P, HG]))
            bias_row[nm] = t
        twos = const.tile([P, 1], F32)
        nc.vector.memset(twos, 2.0)

        # ---- persistent activations ----
        QT = persist.tile([P, 4, S], F32R)       # [j%128, jt, s]
        KT = persist.tile([P, 4, TKS], F32R)
        KQT = persist.tile([P, 4, S], F32R)
        KKT = persist.tile([P, 4, TKK], F32R)
        Vaug = persist.tile([P, TKS // P, NHL, HD + 1], F32R)   # [t%128, tt, h, d|2]
        KVaug = persist.tile([P, TKK // P, NHL, HD + 1], F32R)
        # output staging in two head-halves so the first DMA overlaps the tail
        out_half = [persist.tile([P, S // P, 4, HD], F32, name=f"out_half{i}",
                                 tag=f"out_half{i}") for i in range(2)]

        hsT = stage.tile([P, HC, S], F32R)       # [h%128, hc, s]
        ehsT = stage.tile([P, HC, TKK], F32R)

        # ---- stage-0 emitter: transpose one 128-row tile of hs/ehs ----
        def tp_tile(src, dstT, st, tag, bufs):
            h_tile = attp.tile([P, H], F32, name=f"h_stage_{tag}", tag=tag,
                               bufs=bufs)
            nc.sync.dma_start(out=h_tile, in_=src[st * P:(st + 1) * P, :])
            for hc0 in range(0, HC, 4):
                tp = psproj.tile([P, 4, P], F32, name="tp0", tag="psj")
                for i in range(4):
                    nc.tensor.transpose(
                        tp[:, i, :],
                        h_tile[:, (hc0 + i) * P:(hc0 + i + 1) * P], ident)
                nc.vector.tensor_copy(
                    dstT[:, hc0:hc0 + 4, st * P:(st + 1) * P], tp)

        # ---- projection emitters ----
        def proj_t_chunk(nm, dst, srcT, skeys, jt):
            wjt = wpool.tile([P, HC, P], F32R, name=f"w_{nm}_{jt}", tag="w")
            nc.sync.dma_start(
                out=wjt,
                in_=w_in[nm][:, jt * P:(jt + 1) * P].rearrange(
                    "(hc p) j -> p hc j", p=P))
            for sc in range(skeys // 512):
                ps = psproj.tile([P, 512], F32, name="psj", tag="psj")
                for hc in range(HC):
                    nc.tensor.matmul(
                        ps, lhsT=wjt[:, hc, :],
                        rhs=srcT[:, hc, sc * 512:(sc + 1) * 512],
                        start=(hc == 0), stop=(hc == HC - 1))
                nc.vector.tensor_scalar_add(
                    dst[:, jt, sc * 512:(sc + 1) * 512], ps,
                    bias_col[nm][:, jt:jt + 1])

        def proj_v_load(nm):
            wvs = []
            for hc in range(HC):
                wv = wvpool.tile([P, 512], F32R, name=f"wv_{nm}_{hc}", tag="wv")
                nc.sync.dma_start(out=wv, in_=w_in[nm][hc * P:(hc + 1) * P, :])
                wvs.append(wv)
            return wvs

        def proj_v_chunk(nm, dst, srcT, wvs, tt):
            ps = psproj.tile([P, 512], F32, name=f"psv{tt}", tag="psj")
            for hc in range(HC):
                nc.tensor.matmul(
                    ps, lhsT=srcT[:, hc, tt * P:(tt + 1) * P],
                    rhs=wvs[hc], start=(hc == 0), stop=(hc == HC - 1))
            nc.vector.scalar_tensor_tensor(
                out=dst[:, tt, :, 0:HD],
                in0=ps.rearrange("p (h d) -> p h d", h=NHL),
                scalar=1.0,
                in1=bias_row[nm].rearrange("p (h d) -> p h d", h=NHL),
                op0=ALU.mult, op1=ALU.add)
            nc.vector.tensor_copy(
                dst[:, tt, :, HD:HD + 1],
                twos.unsqueeze(1).broadcast_to([P, NHL, 1]))

        # ---- hs transposes woven with the V projection, then jt0 of Q/K ----
        wvs_v = None
        for st in range(S // P):
            tp_tile(hs, hsT, st, "E", 2)
            if wvs_v is None:
                wvs_v = proj_v_load("v")
            proj_v_chunk("v", Vaug, hsT, wvs_v, st)
        proj_t_chunk("q", QT, hsT, S, 0)
        proj_t_chunk("k", KT, hsT, TKS, 0)

        # ---- attention with interleaved remaining projections ----
        psbig = ctx.enter_context(tc.tile_pool(name="psbig", bufs=3, space="PSUM"))

        def head_branch(h, kt_mat, q_mat, vaug, n_keys, msk):
            base = (h % 2) * HD
            jt = h // 2
            ctx_ps = psbig.tile([65, S], F32, name=f"ctx_{h}_{n_keys}", tag="big")
            nkt = n_keys // P
            for kt in range(nkt):
                st_ps = psbig.tile([P, S], F32, name=f"st_{h}_{kt}", tag="big")
                for sc in range(S // 512):
                    nc.tensor.matmul(
                        st_ps[:, sc * 512:(sc + 1) * 512],
                        lhsT=kt_mat[base:base + HD, jt, kt * P:(kt + 1) * P],
                        rhs=q_mat[base:base + HD, jt, sc * 512:(sc + 1) * 512],
                        start=True, stop=True)
                e_sb = attp.tile([P, S], F32R, name=f"e_{h}_{kt}", tag="E", bufs=2)
                nc.scalar.activation(e_sb, st_ps, AF.Exp,
                                     bias=msk[:, kt:kt + 1], scale=INV)
                if _DUMP and h == 0 and kt == 0:
                    dtag = "s" if n_keys == TKS else "k"
                    d = nc.dram_tensor(f"d_e0{dtag}", [P, S], F32,
                                       kind="ExternalOutput")
                    nc.sync.dma_start(out=d.ap(), in_=e_sb.bitcast(F32))
                for sc in range(S // 512):
                    nc.tensor.matmul(
                        ctx_ps[:, sc * 512:(sc + 1) * 512],
                        lhsT=vaug[:, kt, h, :],
                        rhs=e_sb[:, sc * 512:(sc + 1) * 512],
                        start=(kt == 0), stop=(kt == nkt - 1))
            ctx_sb = ctxp.tile([65, S], F32, name=f"ctxsb_{h}_{n_keys}",
                               tag="ctx_sb", bufs=4)
            nc.vector.tensor_copy(ctx_sb, ctx_ps)
            if _DUMP and h == 0:
                dtag = "s" if n_keys == TKS else "k"
                d = nc.dram_tensor(f"d_ctx{dtag}", [65, S], F32,
                                   kind="ExternalOutput")
                nc.sync.dma_start(out=d.ap(), in_=ctx_sb)
            return ctx_sb

        def post(h, ctx_self, ctx_knl):
            oh = out_half[h // 4]
            for half in range(2):   # s-chunk groups 0-3 / 4-7
                tA = psbig.tile([P, 4, HD + 1], F32, name=f"tA_{h}_{half}",
                                tag="big")
                tB = psbig.tile([P, 4, HD + 1], F32, name=f"tB_{h}_{half}",
                                tag="big")
                for i in range(4):
                    sc = half * 4 + i
                    ssl = slice(sc * P, (sc + 1) * P)
                    nc.tensor.transpose(tA[:, i, :], ctx_self[:, ssl],
                                        ident[0:HD + 1, 0:HD + 1])
                    nc.tensor.transpose(tB[:, i, :], ctx_knl[:, ssl],
                                        ident[0:HD + 1, 0:HD + 1])
                rbs = ctxp.tile([P, 4, 1], F32, name=f"rbs_{h}_{half}", tag="rbs")
                rbk = ctxp.tile([P, 4, 1], F32, name=f"rbk_{h}_{half}", tag="rbk")
                nc.vector.reciprocal(rbs, tA[:, :, HD:HD + 1])
                nc.vector.reciprocal(rbk, tB[:, :, HD:HD + 1])
                msf = ctxp.tile([P, 4, HD], F32, name=f"msf_{h}_{half}", tag="msf")
                mkn = ctxp.tile([P, 4, HD], F32, name=f"mkn_{h}_{half}", tag="mkn")
                nc.vector.tensor_tensor(
                    out=msf, in0=tA[:, :, 0:HD],
                    in1=rbs.broadcast_to([P, 4, HD]), op=ALU.mult)
                nc.vector.tensor_tensor(
                    out=mkn, in0=tB[:, :, 0:HD],
                    in1=rbk.broadcast_to([P, 4, HD]), op=ALU.mult)
                nc.vector.tensor_tensor(
                    out=oh[:, half * 4:half * 4 + 4, h % 4, :],
                    in0=msf, in1=mkn, op=ALU.add)

        def self_branch(h):
            return head_branch(h, KT, QT, Vaug, TKS, mask_sb)

        def knl_branch(h):
            return head_branch(h, KKT, KQT, KVaug, TKK, emask_sb)

        def out_dma(i):
            nc.sync.dma_start(
                out=out.ap()[:, i * 256:(i + 1) * 256].rearrange(
                    "(sc p) j -> p sc j", p=P),
                in_=out_half[i].rearrange("p sc h d -> p sc (h d)"))

        ctx_self = {}
        ctx_knl = {}
        wvs_kv = None

        def fill(h):
            nonlocal wvs_kv
            if h == 0:
                for st in range(TKK // P):
                    tp_tile(ehs, ehsT, st, "ehst", 1)
                wvs_kv = proj_v_load("kv")
            elif h == 1:
                for tt in range(TKK // P):
                    proj_v_chunk("kv", KVaug, ehsT, wvs_kv, tt)
                proj_t_chunk("q", QT, hsT, S, 1)
                proj_t_chunk("k", KT, hsT, TKS, 1)
            elif h == 2:
                proj_t_chunk("kq", KQT, hsT, S, 0)
                proj_t_chunk("kk", KKT, ehsT, TKK, 0)
            elif h == 3:
                proj_t_chunk("q", QT, hsT, S, 2)
                proj_t_chunk("k", KT, hsT, TKS, 2)
                proj_t_chunk("kq", KQT, hsT, S, 1)
            elif h == 4:
                proj_t_chunk("kk", KKT, ehsT, TKK, 1)
                proj_t_chunk("q", QT, hsT, S, 3)
                proj_t_chunk("k", KT, hsT, TKS, 3)
            elif h == 5:
                proj_t_chunk("kq", KQT, hsT, S, 2)
                proj_t_chunk("kk", KKT, ehsT, TKK, 2)
            elif h == 6:
                proj_t_chunk("kq", KQT, hsT, S, 3)
                proj_t_chunk("kk", KKT, ehsT, TKK, 3)

        for h in range(NHL):
            ctx_self[h] = self_branch(h)
            fill(h)
            if h >= 2:
                hp = h - 2
                ctx_knl[hp] = knl_branch(hp)
                post(hp, ctx_self.pop(hp), ctx_knl.pop(hp))
                if hp == 3:
                    out_dma(0)
        for hp in range(NHL - 2, NHL):
            ctx_knl[hp] = knl_branch(hp)
            post(hp, ctx_self.pop(hp), ctx_knl.pop(hp))
        out_dma(1)

        if _DUMP:
            for nm, t in [("d_QT", QT), ("d_KT", KT),
                          ("d_KQT", KQT), ("d_KKT", KKT), ("d_Vaug", Vaug),
                          ("d_KVaug", KVaug), ("d_hsT", hsT)]:
                d = nc.dram_tensor(nm, list(t.shape), F32, kind="ExternalOutput")
                nc.sync.dma_start(out=d.ap(), in_=t.bitcast(F32) if t.dtype == F32R else t)

    nc.finalize()
    return nc


def _get_nc():
    if "nc" not in _CACHE:
        _CACHE["nc"] = _build()
    return _CACHE["nc"]


def kernel(**inputs):
    inp = {k: np.asarray(v, dtype=np.float32) for k, v in inputs.items()}
    nc = _get_nc()

    B = 4
    in_maps = []
    for core in range(8):
        b, hg = core // 2, core % 2
        sl = slice(hg * HG, (hg + 1) * HG)
        m = {
            "hs": np.ascontiguousarray(inp["hidden_states"][b]),
            "ehs": np.ascontiguousarray(inp["encoder_hidden_states"][b]),
            "mask": np.ascontiguousarray(inp["attention_mask"][b, 0, 0, :]),
            "emask": np.ascontiguousarray(inp["encoder_attention_mask"][b, 0, 0, :]),
        }
        for nm in ["q", "k", "v", "kq", "kk", "kv"]:
            m[f"w{nm}"] = np.ascontiguousarray(inp[f"W{nm}"][:, sl])
            m[f"b{nm}"] = np.ascontiguousarray(inp[f"b{nm}"][sl])
        in_maps.append(m)

    res = run_bass_kernel_spmd(nc, in_maps, core_ids=list(range(8)))

    outp = np.empty((B, S, H), np.float32)
    for core in range(8):
        b, hg = core // 2, core % 2
        outp[b, :, hg * HG:(hg + 1) * HG] = res.results[core]["out"]
    return outp


# revision 33
# speedup vs baseline: 1.0095x; 1.0095x over previous
"""Trainium2 Bass kernel for nn_BertSelfAttention_7962869367489.

Dual-branch (self + cross/"knowledge") BERT attention, B=4, S=1024, K=512,
H=1024, NH=16, HD=64, fp32.

Sharding: 8 cores = (batch b in 0..3) x (head-group hg in 0..1, 8 heads each).
All six projections are column-split by head-group; per-head attention is
entirely core-local; output columns are disjoint per core, so the gather is a
pure concatenation (no collectives).

Per-core pipeline (matmul operands in f32r = TF32-like single-pass PE mode,
~1.5e-4 rel err, full PE rate at free-dim >= 256):
  1. PE-transpose hs -> hsT [H, S] woven with the V projection; ehs -> ehsT
     transposed inside the first attention fill slot.
  2. Projections: QT/KT/KQT = W.T @ hsT (+bias), KKT = Wkk.T @ ehsT
     (transposed outputs); Vaug/KVaug = hs @ Wv in normal orientation with an
     augmented column of 2.0 (the ctxT matmul then also produces
     2*softmax-denominator, folding the (ctx+kctx)*0.5 average into the
     normalization for free).
  3. Per head h: scoresT[t,s] = K_h @ Q_h^T via lhsT=KT_h, rhs=QT_h
     (contraction HD=64); exp on ACT with per-partition mask bias and 1/8
     scale; ctxT_aug[65,S] += Vaug_h^T @ E accumulated over key chunks;
     PE-transpose back to [S, 64|den]; normalize + merge both branches on DVE.
  Remaining projections are interleaved between attention branches so the
  PE-heavy projection work fills the ACT-bound (exp) gaps; knowledge branches
  run delayed by 2 heads; output is DMA'd in two head-halves.
"""
import numpy as np
from contextlib import ExitStack

import concourse.bacc as bacc
import concourse.tile as tile
import concourse.mybir as mybir
from concourse.bass_utils import run_bass_kernel_spmd

F32 = mybir.dt.float32
F32R = mybir.dt.float32r
AF = mybir.ActivationFunctionType
ALU = mybir.AluOpType

P = 128
S = 1024        # query length
TKS = 1024      # self-branch key length
TKK = 512       # knowledge-branch key length
H = 1024        # model dim (projection contraction)
HG = 512        # per-core output width (8 heads x 64)
NHL = 8         # heads per core
HD = 64
HC = H // P     # 8 contraction chunks
INV = 0.125     # 1/sqrt(64)

_CACHE = {}
_DUMP = False


def _build():
    nc = bacc.Bacc(target_bir_lowering=False, debug=False)

    hs = nc.dram_tensor("hs", [S, H], F32, kind="ExternalInput")
    ehs = nc.dram_tensor("ehs", [TKK, H], F32, kind="ExternalInput")
    w_in = {}
    b_in = {}
    for nm in ["q", "k", "v", "kq", "kk", "kv"]:
        w_in[nm] = nc.dram_tensor(f"w{nm}", [H, HG], F32R, kind="ExternalInput")
        b_in[nm] = nc.dram_tensor(f"b{nm}", [HG], F32, kind="ExternalInput")
    mask = nc.dram_tensor("mask", [TKS], F32, kind="ExternalInput")
    emask = nc.dram_tensor("emask", [TKK], F32, kind="ExternalInput")
    out = nc.dram_tensor("out", [S, HG], F32, kind="ExternalOutput")

    with tile.TileContext(nc) as tc, ExitStack() as ctx:
        const = ctx.enter_context(tc.tile_pool(name="const", bufs=1))
        persist = ctx.enter_context(tc.tile_pool(name="persist", bufs=1))
        stage = ctx.enter_context(tc.tile_pool(name="stage", bufs=1))
        wpool = ctx.enter_context(tc.tile_pool(name="wpool", bufs=2))
        wvpool = ctx.enter_context(tc.tile_pool(name="wvpool", bufs=8))
        attp = ctx.enter_context(tc.tile_pool(name="att", bufs=3))
        ctxp = ctx.enter_context(tc.tile_pool(name="ctxp", bufs=2))
        psproj = ctx.enter_context(tc.tile_pool(name="psproj", bufs=2, space="PSUM"))

        # ---- constants ----
        ident_dram = nc.inline_tensor(np.eye(P, dtype=np.float32), name="ident_c")
        ident = const.tile([P, P], F32)
        nc.sync.dma_start(out=ident, in_=ident_dram.ap())
        mask_sb = const.tile([P, TKS // P], F32)
        nc.gpsimd.dma_start(out=mask_sb, in_=mask.ap().rearrange("(kt p) -> p kt", p=P))
        emask_sb = const.tile([P, TKK // P], F32)
        nc.gpsimd.dma_start(out=emask_sb, in_=emask.ap().rearrange("(kt p) -> p kt", p=P))
        bias_col = {}
        for nm in ["q", "k", "kq", "kk"]:
            t = const.tile([P, 4], F32, name=f"bias_{nm}")
            nc.gpsimd.dma_start(out=t, in_=b_in[nm].ap().rearrange("(jt p) -> p jt", p=P))
            bias_col[nm] = t
        bias_row = {}
        for nm in ["v", "kv"]:
            t = const.tile([P, HG], F32, name=f"brow_{nm}")
            nc.gpsimd.dma_start(out=t, in_=b_in[nm].ap().unsqueeze(0).broadcast_to([P, HG]))
            bias_row[nm] = t
        twos = const.tile([P, 1], F32)
        nc.vector.memset(twos, 2.0)

        # ---- persistent activations ----
        QT = persist.tile([P, 4, S], F32R)       # [j%128, jt, s]
        KT = persist.tile([P, 4, TKS], F32R)
        KQT = persist.tile([P, 4, S], F32R)
        KKT = persist.tile([P, 4, TKK], F32R)
        Vaug = persist.tile([P, TKS // P, NHL, HD + 1], F32R)   # [t%128, tt, h, d|2]
        KVaug = persist.tile([P, TKK // P, NHL, HD + 1], F32R)
        # output staging in two head-halves so the first DMA overlaps the tail
        out_half = [persist.tile([P, S // P, 4, HD], F32, name=f"out_half{i}",
                                 tag=f"out_half{i}") for i in range(2)]

        hsT = stage.tile([P, HC, S], F32R)       # [h%128, hc, s]
        ehsT = stage.tile([P, HC, TKK], F32R)

        # ---- stage-0 emitter: transpose one 128-row tile of hs/ehs ----
        def tp_tile(src, dstT, st, tag, bufs):
            h_tile = attp.tile([P, H], F32, name=f"h_stage_{tag}", tag=tag,
                               bufs=bufs)
            # split the staging load so the first transposes start at half-load
            nc.sync.dma_start(out=h_tile[:, 0:H // 2],
                              in_=src[st * P:(st + 1) * P, 0:H // 2])
            nc.sync.dma_start(out=h_tile[:, H // 2:H],
                              in_=src[st * P:(st + 1) * P, H // 2:H])
            for hc0 in range(0, HC, 4):
                tp = psproj.tile([P, 4, P], F32, name="tp0", tag="psj")
                for i in range(4):
                    nc.tensor.transpose(
                        tp[:, i, :],
                        h_tile[:, (hc0 + i) * P:(hc0 + i + 1) * P], ident)
                nc.vector.tensor_copy(
                    dstT[:, hc0:hc0 + 4, st * P:(st + 1) * P], tp)

        # ---- projection emitters ----
        def proj_t_chunk(nm, dst, srcT, skeys, jt):
            wjt = wpool.tile([P, HC, P], F32R, name=f"w_{nm}_{jt}", tag="w")
            nc.sync.dma_start(
                out=wjt,
                in_=w_in[nm][:, jt * P:(jt + 1) * P].rearrange(
                    "(hc p) j -> p hc j", p=P))
            for sc in range(skeys // 512):
                ps = psproj.tile([P, 512], F32, name="psj", tag="psj")
                for hc in range(HC):
                    nc.tensor.matmul(
                        ps, lhsT=wjt[:, hc, :],
                        rhs=srcT[:, hc, sc * 512:(sc + 1) * 512],
                        start=(hc == 0), stop=(hc == HC - 1))
                nc.vector.tensor_scalar_add(
                    dst[:, jt, sc * 512:(sc + 1) * 512], ps,
                    bias_col[nm][:, jt:jt + 1])

        def proj_v_load(nm):
            wvs = []
            for hc in range(HC):
                wv = wvpool.tile([P, 512], F32R, name=f"wv_{nm}_{hc}", tag="wv")
                nc.sync.dma_start(out=wv, in_=w_in[nm][hc * P:(hc + 1) * P, :])
                wvs.append(wv)
            return wvs

        def proj_v_chunk(nm, dst, srcT, wvs, tt):
            ps = psproj.tile([P, 512], F32, name=f"psv{tt}", tag="psj")
            for hc in range(HC):
                nc.tensor.matmul(
                    ps, lhsT=srcT[:, hc, tt * P:(tt + 1) * P],
                    rhs=wvs[hc], start=(hc == 0), stop=(hc == HC - 1))
            nc.vector.scalar_tensor_tensor(
                out=dst[:, tt, :, 0:HD],
                in0=ps.rearrange("p (h d) -> p h d", h=NHL),
                scalar=1.0,
                in1=bias_row[nm].rearrange("p (h d) -> p h d", h=NHL),
                op0=ALU.mult, op1=ALU.add)
            nc.vector.tensor_copy(
                dst[:, tt, :, HD:HD + 1],
                twos.unsqueeze(1).broadcast_to([P, NHL, 1]))

        # ---- hs transposes woven with the V projection, then jt0 of Q/K ----
        wvs_v = None
        for st in range(S // P):
            tp_tile(hs, hsT, st, "E", 2)
            if wvs_v is None:
                wvs_v = proj_v_load("v")
            proj_v_chunk("v", Vaug, hsT, wvs_v, st)
        proj_t_chunk("q", QT, hsT, S, 0)
        proj_t_chunk("k", KT, hsT, TKS, 0)

        # ---- attention with interleaved remaining projections ----
        psbig = ctx.enter_context(tc.tile_pool(name="psbig", bufs=3, space="PSUM"))

        def head_branch(h, kt_mat, q_mat, vaug, n_keys, msk):
            base = (h % 2) * HD
            jt = h // 2
            ctx_ps = psbig.tile([65, S], F32, name=f"ctx_{h}_{n_keys}", tag="big")
            nkt = n_keys // P
            for kt in range(nkt):
                st_ps = psbig.tile([P, S], F32, name=f"st_{h}_{kt}", tag="big")
                for sc in range(S // 512):
                    nc.tensor.matmul(
                        st_ps[:, sc * 512:(sc + 1) * 512],
                        lhsT=kt_mat[base:base + HD, jt, kt * P:(kt + 1) * P],
                        rhs=q_mat[base:base + HD, jt, sc * 512:(sc + 1) * 512],
                        start=True, stop=True)
                e_sb = attp.tile([P, S], F32R, name=f"e_{h}_{kt}", tag="E", bufs=2)
                nc.scalar.activation(e_sb, st_ps, AF.Exp,
                                     bias=msk[:, kt:kt + 1], scale=INV)
                if _DUMP and h == 0 and kt == 0:
                    dtag = "s" if n_keys == TKS else "k"
                    d = nc.dram_tensor(f"d_e0{dtag}", [P, S], F32,
                                       kind="ExternalOutput")
                    nc.sync.dma_start(out=d.ap(), in_=e_sb.bitcast(F32))
                for sc in range(S // 512):
                    nc.tensor.matmul(
                        ctx_ps[:, sc * 512:(sc + 1) * 512],
                        lhsT=vaug[:, kt, h, :],
                        rhs=e_sb[:, sc * 512:(sc + 1) * 512],
                        start=(kt == 0), stop=(kt == nkt - 1))
            ctx_sb = ctxp.tile([65, S], F32, name=f"ctxsb_{h}_{n_keys}",
                               tag="ctx_sb", bufs=4)
            nc.vector.tensor_copy(ctx_sb, ctx_ps)
            if _DUMP and h == 0:
                dtag = "s" if n_keys == TKS else "k"
                d = nc.dram_tensor(f"d_ctx{dtag}", [65, S], F32,
                                   kind="ExternalOutput")
                nc.sync.dma_start(out=d.ap(), in_=ctx_sb)
            return ctx_sb

        def post(h, ctx_self, ctx_knl):
            oh = out_half[h // 4]
            for half in range(2):   # s-chunk groups 0-3 / 4-7
                tA = psbig.tile([P, 4, HD + 1], F32, name=f"tA_{h}_{half}",
                                tag="big")
                tB = psbig.tile([P, 4, HD + 1], F32, name=f"tB_{h}_{half}",
                                tag="big")
                for i in range(4):
                    sc = half * 4 + i
                    ssl = slice(sc * P, (sc + 1) * P)
                    nc.tensor.transpose(tA[:, i, :], ctx_self[:, ssl],
                                        ident[0:HD + 1, 0:HD + 1])
                    nc.tensor.transpose(tB[:, i, :], ctx_knl[:, ssl],
                                        ident[0:HD + 1, 0:HD + 1])
                rbs = ctxp.tile([P, 4, 1], F32, name=f"rbs_{h}_{half}", tag="rbs")
                rbk = ctxp.tile([P, 4, 1], F32, name=f"rbk_{h}_{half}", tag="rbk")
                nc.vector.reciprocal(rbs, tA[:, :, HD:HD + 1])
                nc.vector.reciprocal(rbk, tB[:, :, HD:HD + 1])
                msf = ctxp.tile([P, 4, HD], F32, name=f"msf_{h}_{half}", tag="msf", bufs=1)
                mkn = ctxp.tile([P, 4, HD], F32, name=f"mkn_{h}_{half}", tag="mkn", bufs=1)
                nc.vector.tensor_tensor(
                    out=msf, in0=tA[:, :, 0:HD],
                    in1=rbs.broadcast_to([P, 4, HD]), op=ALU.mult)
                nc.vector.tensor_tensor(
                    out=mkn, in0=tB[:, :, 0:HD],
                    in1=rbk.broadcast_to([P, 4, HD]), op=ALU.mult)
                nc.vector.tensor_tensor(
                    out=oh[:, half * 4:half * 4 + 4, h % 4, :],
                    in0=msf, in1=mkn, op=ALU.add)

        def self_branch(h):
            return head_branch(h, KT, QT, Vaug, TKS, mask_sb)

        def knl_branch(h):
            return head_branch(h, KKT, KQT, KVaug, TKK, emask_sb)

        def out_dma(i):
            nc.sync.dma_start(
                out=out.ap()[:, i * 256:(i + 1) * 256].rearrange(
                    "(sc p) j -> p sc j", p=P),
                in_=out_half[i].rearrange("p sc h d -> p sc (h d)"))

        ctx_self = {}
        ctx_knl = {}
        wvs_kv = None

        def fill(h):
            nonlocal wvs_kv
            if h == 0:
                for st in range(TKK // P):
                    tp_tile(ehs, ehsT, st, "ehst", 1)
                wvs_kv = proj_v_load("kv")
            elif h == 1:
                for tt in range(TKK // P):
                    proj_v_chunk("kv", KVaug, ehsT, wvs_kv, tt)
                proj_t_chunk("q", QT, hsT, S, 1)
                proj_t_chunk("k", KT, hsT, TKS, 1)
            elif h == 2:
                proj_t_chunk("kq", KQT, hsT, S, 0)
                proj_t_chunk("kk", KKT, ehsT, TKK, 0)
            elif h == 3:
                proj_t_chunk("q", QT, hsT, S, 2)
                proj_t_chunk("k", KT, hsT, TKS, 2)
                proj_t_chunk("kq", KQT, hsT, S, 1)
            elif h == 4:
                proj_t_chunk("kk", KKT, ehsT, TKK, 1)
                proj_t_chunk("q", QT, hsT, S, 3)
                proj_t_chunk("k", KT, hsT, TKS, 3)
            elif h == 5:
                proj_t_chunk("kq", KQT, hsT, S, 2)
                proj_t_chunk("kk", KKT, ehsT, TKK, 2)
            elif h == 6:
                proj_t_chunk("kq", KQT, hsT, S, 3)
                proj_t_chunk("kk", KKT, ehsT, TKK, 3)

        for h in range(NHL):
            ctx_self[h] = self_branch(h)
            fill(h)
            if h >= 2:
                hp = h - 2
                ctx_knl[hp] = knl_branch(hp)
                post(hp, ctx_self.pop(hp), ctx_knl.pop(hp))
                if hp == 3:
                    out_dma(0)
        for hp in range(NHL - 2, NHL):
            ctx_knl[hp] = knl_branch(hp)
            post(hp, ctx_self.pop(hp), ctx_knl.pop(hp))
        out_dma(1)

        if _DUMP:
            for nm, t in [("d_QT", QT), ("d_KT", KT),
                          ("d_KQT", KQT), ("d_KKT", KKT), ("d_Vaug", Vaug),
                          ("d_KVaug", KVaug), ("d_hsT", hsT)]:
                d = nc.dram_tensor(nm, list(t.shape), F32, kind="ExternalOutput")
                nc.sync.dma_start(out=d.ap(), in_=t.bitcast(F32) if t.dtype == F32R else t)

    nc.finalize()
    return nc


def _get_nc():
    if "nc" not in _CACHE:
        _CACHE["nc"] = _build()
    return _CACHE["nc"]


def kernel(**inputs):
    inp = {k: np.asarray(v, dtype=np.float32) for k, v in inputs.items()}
    nc = _get_nc()

    B = 4
    in_maps = []
    for core in range(8):
        b, hg = core // 2, core % 2
        sl = slice(hg * HG, (hg + 1) * HG)
        m = {
            "hs": np.ascontiguousarray(inp["hidden_states"][b]),
            "ehs": np.ascontiguousarray(inp["encoder_hidden_states"][b]),
            "mask": np.ascontiguousarray(inp["attention_mask"][b, 0, 0, :]),
            "emask": np.ascontiguousarray(inp["encoder_attention_mask"][b, 0, 0, :]),
        }
        for nm in ["q", "k", "v", "kq", "kk", "kv"]:
            m[f"w{nm}"] = np.ascontiguousarray(inp[f"W{nm}"][:, sl])
            m[f"b{nm}"] = np.ascontiguousarray(inp[f"b{nm}"][sl])
        in_maps.append(m)

    res = run_bass_kernel_spmd(nc, in_maps, core_ids=list(range(8)))

    outp = np.empty((B, S, H), np.float32)
    for core in range(8):
        b, hg = core // 2, core % 2
        outp[b, :, hg * HG:(hg + 1) * HG] = res.results[core]["out"]
    return outp


# revision 34
# speedup vs baseline: 1.0605x; 1.0505x over previous
"""Trainium2 Bass kernel for nn_BertSelfAttention_7962869367489.

Dual-branch (self + cross/"knowledge") BERT attention, B=4, S=1024, K=512,
H=1024, NH=16, HD=64, fp32.

Sharding: 8 cores = (batch b in 0..3) x (head-group hg in 0..1, 8 heads each).
All six projections are column-split by head-group; per-head attention is
entirely core-local; output columns are disjoint per core, so the gather is a
pure concatenation (no collectives).

Per-core pipeline (matmul operands in f32r = TF32-like single-pass PE mode,
~1.5e-4 rel err, full PE rate at free-dim >= 256):
  1. PE-transpose hs -> hsT [H, S] woven with the V projection; ehs -> ehsT
     transposed inside the first attention fill slot.
  2. Projections: QT/KT/KQT = W.T @ hsT (+bias), KKT = Wkk.T @ ehsT
     (transposed outputs); Vaug/KVaug = hs @ Wv in normal orientation with an
     augmented column of 2.0 (the ctxT matmul then also produces
     2*softmax-denominator, folding the (ctx+kctx)*0.5 average into the
     normalization for free).
  3. Per head h: scoresT[t,s] = K_h @ Q_h^T via lhsT=KT_h, rhs=QT_h
     (contraction HD=64); exp on ACT with per-partition mask bias and 1/8
     scale; ctxT_aug[65,S] += Vaug_h^T @ E accumulated over key chunks;
     PE-transpose back to [S, 64|den]; normalize + merge both branches on DVE.
  Remaining projections are interleaved between attention branches so the
  PE-heavy projection work fills the ACT-bound (exp) gaps; knowledge branches
  run delayed by 2 heads; output is DMA'd in two head-halves.
"""
import numpy as np
from contextlib import ExitStack

import concourse.bacc as bacc
import concourse.tile as tile
import concourse.mybir as mybir
from concourse.bass_utils import run_bass_kernel_spmd

F32 = mybir.dt.float32
F32R = mybir.dt.float32r
AF = mybir.ActivationFunctionType
ALU = mybir.AluOpType

P = 128
S = 1024        # query length
TKS = 1024      # self-branch key length
TKK = 512       # knowledge-branch key length
H = 1024        # model dim (projection contraction)
HG = 512        # per-core output width (8 heads x 64)
NHL = 8         # heads per core
HD = 64
HC = H // P     # 8 contraction chunks
INV = 0.125     # 1/sqrt(64)

_CACHE = {}
_DUMP = False


def _build():
    nc = bacc.Bacc(target_bir_lowering=False, debug=False)

    hs = nc.dram_tensor("hs", [S, H], F32, kind="ExternalInput")
    ehs = nc.dram_tensor("ehs", [TKK, H], F32, kind="ExternalInput")
    w_in = {}
    b_in = {}
    for nm in ["q", "k", "v", "kq", "kk", "kv"]:
        w_in[nm] = nc.dram_tensor(f"w{nm}", [H, HG], F32R, kind="ExternalInput")
        b_in[nm] = nc.dram_tensor(f"b{nm}", [HG], F32, kind="ExternalInput")
    mask = nc.dram_tensor("mask", [TKS], F32, kind="ExternalInput")
    emask = nc.dram_tensor("emask", [TKK], F32, kind="ExternalInput")
    out = nc.dram_tensor("out", [S, HG], F32, kind="ExternalOutput")

    with tile.TileContext(nc) as tc, ExitStack() as ctx:
        const = ctx.enter_context(tc.tile_pool(name="const", bufs=1))
        persist = ctx.enter_context(tc.tile_pool(name="persist", bufs=1))
        stage = ctx.enter_context(tc.tile_pool(name="stage", bufs=1))
        wpool = ctx.enter_context(tc.tile_pool(name="wpool", bufs=2))
        wvpool = ctx.enter_context(tc.tile_pool(name="wvpool", bufs=8))
        attp = ctx.enter_context(tc.tile_pool(name="att", bufs=3))
        ctxp = ctx.enter_context(tc.tile_pool(name="ctxp", bufs=2))
        psproj = ctx.enter_context(tc.tile_pool(name="psproj", bufs=2, space="PSUM"))

        # ---- constants ----
        ident_dram = nc.inline_tensor(np.eye(P, dtype=np.float32), name="ident_c")
        ident = const.tile([P, P], F32)
        nc.sync.dma_start(out=ident, in_=ident_dram.ap())
        mask_sb = const.tile([P, TKS // P], F32)
        nc.gpsimd.dma_start(out=mask_sb, in_=mask.ap().rearrange("(kt p) -> p kt", p=P))
        emask_sb = const.tile([P, TKK // P], F32)
        nc.gpsimd.dma_start(out=emask_sb, in_=emask.ap().rearrange("(kt p) -> p kt", p=P))
        bias_col = {}
        for nm in ["q", "k", "kq", "kk"]:
            t = const.tile([P, 4], F32, name=f"bias_{nm}")
            nc.gpsimd.dma_start(out=t, in_=b_in[nm].ap().rearrange("(jt p) -> p jt", p=P))
            bias_col[nm] = t
        bias_row = {}
        for nm in ["v", "kv"]:
            t = const.tile([P, HG], F32, name=f"brow_{nm}")
            nc.gpsimd.dma_start(out=t, in_=b_in[nm].ap().unsqueeze(0).broadcast_to([P, HG]))
            bias_row[nm] = t
        twos = const.tile([P, 1], F32)
        nc.vector.memset(twos, 2.0)

        # ---- persistent activations ----
        QT = persist.tile([P, 4, S], F32R)       # [j%128, jt, s]
        KT = persist.tile([P, 4, TKS], F32R)
        KQT = persist.tile([P, 4, S], F32R)
        KKT = persist.tile([P, 4, TKK], F32R)
        Vaug = persist.tile([P, TKS // P, NHL, HD + 1], F32R)   # [t%128, tt, h, d|2]
        KVaug = persist.tile([P, TKK // P, NHL, HD + 1], F32R)
        # output staging in two head-halves so the first DMA overlaps the tail
        out_half = [persist.tile([P, S // P, 4, HD], F32, name=f"out_half{i}",
                                 tag=f"out_half{i}") for i in range(2)]

        hsT = stage.tile([P, HC, S], F32R)       # [h%128, hc, s]
        ehsT = stage.tile([P, HC, TKK], F32R)

        # ---- stage-0 emitter: transpose one 128-row tile of hs/ehs ----
        def tp_tile(src, dstT, st, tag, bufs):
            h_tile = attp.tile([P, H], F32, name=f"h_stage_{tag}", tag=tag,
                               bufs=bufs)
            # split the staging load so the first transposes start at half-load
            nc.sync.dma_start(out=h_tile[:, 0:H // 2],
                              in_=src[st * P:(st + 1) * P, 0:H // 2])
            nc.sync.dma_start(out=h_tile[:, H // 2:H],
                              in_=src[st * P:(st + 1) * P, H // 2:H])
            for hc0 in range(0, HC, 4):
                tp = psproj.tile([P, 4, P], F32, name="tp0", tag="psj")
                for i in range(4):
                    nc.tensor.transpose(
                        tp[:, i, :],
                        h_tile[:, (hc0 + i) * P:(hc0 + i + 1) * P], ident)
                nc.vector.tensor_copy(
                    dstT[:, hc0:hc0 + 4, st * P:(st + 1) * P], tp)

        # ---- projection emitters ----
        def proj_t_chunk(nm, dst, srcT, skeys, jt):
            wjt = wpool.tile([P, HC, P], F32R, name=f"w_{nm}_{jt}", tag="w")
            nc.sync.dma_start(
                out=wjt,
                in_=w_in[nm][:, jt * P:(jt + 1) * P].rearrange(
                    "(hc p) j -> p hc j", p=P))
            for sc in range(skeys // 512):
                ps = psproj.tile([P, 512], F32, name="psj", tag="psj")
                for hc in range(HC):
                    nc.tensor.matmul(
                        ps, lhsT=wjt[:, hc, :],
                        rhs=srcT[:, hc, sc * 512:(sc + 1) * 512],
                        start=(hc == 0), stop=(hc == HC - 1))
                nc.vector.tensor_scalar_add(
                    dst[:, jt, sc * 512:(sc + 1) * 512], ps,
                    bias_col[nm][:, jt:jt + 1])

        def proj_v_load(nm):
            wvs = []
            for hc in range(HC):
                wv = wvpool.tile([P, 512], F32R, name=f"wv_{nm}_{hc}", tag="wv")
                nc.sync.dma_start(out=wv, in_=w_in[nm][hc * P:(hc + 1) * P, :])
                wvs.append(wv)
            return wvs

        def proj_v_chunk(nm, dst, srcT, wvs, tt):
            ps = psproj.tile([P, 512], F32, name=f"psv{tt}", tag="psj")
            for hc in range(HC):
                nc.tensor.matmul(
                    ps, lhsT=srcT[:, hc, tt * P:(tt + 1) * P],
                    rhs=wvs[hc], start=(hc == 0), stop=(hc == HC - 1))
            nc.vector.scalar_tensor_tensor(
                out=dst[:, tt, :, 0:HD],
                in0=ps.rearrange("p (h d) -> p h d", h=NHL),
                scalar=1.0,
                in1=bias_row[nm].rearrange("p (h d) -> p h d", h=NHL),
                op0=ALU.mult, op1=ALU.add)
            nc.vector.tensor_copy(
                dst[:, tt, :, HD:HD + 1],
                twos.unsqueeze(1).broadcast_to([P, NHL, 1]))

        # ---- hs transposes woven with the V projection, then jt0 of Q/K ----
        wvs_v = None
        for st in range(S // P):
            tp_tile(hs, hsT, st, "E", 2)
            if wvs_v is None:
                wvs_v = proj_v_load("v")
            proj_v_chunk("v", Vaug, hsT, wvs_v, st)
        proj_t_chunk("q", QT, hsT, S, 0)
        proj_t_chunk("k", KT, hsT, TKS, 0)

        # ---- attention with interleaved remaining projections ----
        psbig = ctx.enter_context(tc.tile_pool(name="psbig", bufs=3, space="PSUM"))

        def head_branch(h, kt_mat, q_mat, vaug, n_keys, msk):
            base = (h % 2) * HD
            jt = h // 2
            ctx_ps = psbig.tile([65, S], F32, name=f"ctx_{h}_{n_keys}", tag="big")
            nkt = n_keys // P

            def ctx_mms(kt, e_sb):
                for sc in range(S // 512):
                    nc.tensor.matmul(
                        ctx_ps[:, sc * 512:(sc + 1) * 512],
                        lhsT=vaug[:, kt, h, :],
                        rhs=e_sb[:, sc * 512:(sc + 1) * 512],
                        start=(kt == 0), stop=(kt == nkt - 1))

            # software-pipelined: ctx(kt-1) is emitted after scores(kt), so the
            # PE overlaps exp(kt-1) latency with the next tile's score matmuls
            pend = None
            for kt in range(nkt):
                st_ps = psbig.tile([P, S], F32, name=f"st_{h}_{kt}", tag="big")
                for sc in range(S // 512):
                    nc.tensor.matmul(
                        st_ps[:, sc * 512:(sc + 1) * 512],
                        lhsT=kt_mat[base:base + HD, jt, kt * P:(kt + 1) * P],
                        rhs=q_mat[base:base + HD, jt, sc * 512:(sc + 1) * 512],
                        start=True, stop=True)
                e_sb = attp.tile([P, S], F32R, name=f"e_{h}_{kt}", tag="E", bufs=2)
                nc.scalar.activation(e_sb, st_ps, AF.Exp,
                                     bias=msk[:, kt:kt + 1], scale=INV)
                if _DUMP and h == 0 and kt == 0:
                    dtag = "s" if n_keys == TKS else "k"
                    d = nc.dram_tensor(f"d_e0{dtag}", [P, S], F32,
                                       kind="ExternalOutput")
                    nc.sync.dma_start(out=d.ap(), in_=e_sb.bitcast(F32))
                if pend is not None:
                    ctx_mms(*pend)
                pend = (kt, e_sb)
            ctx_mms(*pend)
            ctx_sb = ctxp.tile([65, S], F32, name=f"ctxsb_{h}_{n_keys}",
                               tag="ctx_sb", bufs=4)
            nc.vector.tensor_copy(ctx_sb, ctx_ps)
            if _DUMP and h == 0:
                dtag = "s" if n_keys == TKS else "k"
                d = nc.dram_tensor(f"d_ctx{dtag}", [65, S], F32,
                                   kind="ExternalOutput")
                nc.sync.dma_start(out=d.ap(), in_=ctx_sb)
            return ctx_sb

        def post(h, ctx_self, ctx_knl):
            oh = out_half[h // 4]
            for half in range(2):   # s-chunk groups 0-3 / 4-7
                tA = psbig.tile([P, 4, HD + 1], F32, name=f"tA_{h}_{half}",
                                tag="big")
                tB = psbig.tile([P, 4, HD + 1], F32, name=f"tB_{h}_{half}",
                                tag="big")
                for i in range(4):
                    sc = half * 4 + i
                    ssl = slice(sc * P, (sc + 1) * P)
                    nc.tensor.transpose(tA[:, i, :], ctx_self[:, ssl],
                                        ident[0:HD + 1, 0:HD + 1])
                    nc.tensor.transpose(tB[:, i, :], ctx_knl[:, ssl],
                                        ident[0:HD + 1, 0:HD + 1])
                rbs = ctxp.tile([P, 4, 1], F32, name=f"rbs_{h}_{half}", tag="rbs")
                rbk = ctxp.tile([P, 4, 1], F32, name=f"rbk_{h}_{half}", tag="rbk")
                nc.vector.reciprocal(rbs, tA[:, :, HD:HD + 1])
                nc.vector.reciprocal(rbk, tB[:, :, HD:HD + 1])
                msf = ctxp.tile([P, 4, HD], F32, name=f"msf_{h}_{half}", tag="msf", bufs=1)
                mkn = ctxp.tile([P, 4, HD], F32, name=f"mkn_{h}_{half}", tag="mkn", bufs=1)
                nc.vector.tensor_tensor(
                    out=msf, in0=tA[:, :, 0:HD],
                    in1=rbs.broadcast_to([P, 4, HD]), op=ALU.mult)
                nc.vector.tensor_tensor(
                    out=mkn, in0=tB[:, :, 0:HD],
                    in1=rbk.broadcast_to([P, 4, HD]), op=ALU.mult)
                nc.vector.tensor_tensor(
                    out=oh[:, half * 4:half * 4 + 4, h % 4, :],
                    in0=msf, in1=mkn, op=ALU.add)

        def self_branch(h):
            return head_branch(h, KT, QT, Vaug, TKS, mask_sb)

        def knl_branch(h):
            return head_branch(h, KKT, KQT, KVaug, TKK, emask_sb)

        def out_dma(i):
            nc.sync.dma_start(
                out=out.ap()[:, i * 256:(i + 1) * 256].rearrange(
                    "(sc p) j -> p sc j", p=P),
                in_=out_half[i].rearrange("p sc h d -> p sc (h d)"))

        ctx_self = {}
        ctx_knl = {}
        wvs_kv = None

        def fill(h):
            nonlocal wvs_kv
            if h == 0:
                for st in range(TKK // P):
                    tp_tile(ehs, ehsT, st, "ehst", 1)
                wvs_kv = proj_v_load("kv")
            elif h == 1:
                for tt in range(TKK // P):
                    proj_v_chunk("kv", KVaug, ehsT, wvs_kv, tt)
                proj_t_chunk("q", QT, hsT, S, 1)
                proj_t_chunk("k", KT, hsT, TKS, 1)
            elif h == 2:
                proj_t_chunk("kq", KQT, hsT, S, 0)
                proj_t_chunk("kk", KKT, ehsT, TKK, 0)
            elif h == 3:
                proj_t_chunk("q", QT, hsT, S, 2)
                proj_t_chunk("k", KT, hsT, TKS, 2)
                proj_t_chunk("kq", KQT, hsT, S, 1)
            elif h == 4:
                proj_t_chunk("kk", KKT, ehsT, TKK, 1)
                proj_t_chunk("q", QT, hsT, S, 3)
                proj_t_chunk("k", KT, hsT, TKS, 3)
            elif h == 5:
                proj_t_chunk("kq", KQT, hsT, S, 2)
                proj_t_chunk("kk", KKT, ehsT, TKK, 2)
            elif h == 6:
                proj_t_chunk("kq", KQT, hsT, S, 3)
                proj_t_chunk("kk", KKT, ehsT, TKK, 3)

        for h in range(NHL):
            ctx_self[h] = self_branch(h)
            fill(h)
            if h >= 2:
                hp = h - 2
                ctx_knl[hp] = knl_branch(hp)
                post(hp, ctx_self.pop(hp), ctx_knl.pop(hp))
                if hp == 3:
                    out_dma(0)
        for hp in range(NHL - 2, NHL):
            ctx_knl[hp] = knl_branch(hp)
            post(hp, ctx_self.pop(hp), ctx_knl.pop(hp))
        out_dma(1)

        if _DUMP:
            for nm, t in [("d_QT", QT), ("d_KT", KT),
                          ("d_KQT", KQT), ("d_KKT", KKT), ("d_Vaug", Vaug),
                          ("d_KVaug", KVaug), ("d_hsT", hsT)]:
                d = nc.dram_tensor(nm, list(t.shape), F32, kind="ExternalOutput")
                nc.sync.dma_start(out=d.ap(), in_=t.bitcast(F32) if t.dtype == F32R else t)

    nc.finalize()
    return nc


def _get_nc():
    if "nc" not in _CACHE:
        _CACHE["nc"] = _build()
    return _CACHE["nc"]


def kernel(**inputs):
    inp = {k: np.asarray(v, dtype=np.float32) for k, v in inputs.items()}
    nc = _get_nc()

    B = 4
    in_maps = []
    for core in range(8):
        b, hg = core // 2, core % 2
        sl = slice(hg * HG, (hg + 1) * HG)
        m = {
            "hs": np.ascontiguousarray(inp["hidden_states"][b]),
            "ehs": np.ascontiguousarray(inp["encoder_hidden_states"][b]),
            "mask": np.ascontiguousarray(inp["attention_mask"][b, 0, 0, :]),
            "emask": np.ascontiguousarray(inp["encoder_attention_mask"][b, 0, 0, :]),
        }
        for nm in ["q", "k", "v", "kq", "kk", "kv"]:
            m[f"w{nm}"] = np.ascontiguousarray(inp[f"W{nm}"][:, sl])
            m[f"b{nm}"] = np.ascontiguousarray(inp[f"b{nm}"][sl])
        in_maps.append(m)

    res = run_bass_kernel_spmd(nc, in_maps, core_ids=list(range(8)))

    outp = np.empty((B, S, H), np.float32)
    for core in range(8):
        b, hg = core // 2, core % 2
        outp[b, :, hg * HG:(hg + 1) * HG] = res.results[core]["out"]
    return outp


# revision 35
# speedup vs baseline: 1.0817x; 1.0200x over previous
"""Trainium2 Bass kernel for nn_BertSelfAttention_7962869367489.

Dual-branch (self + cross/"knowledge") BERT attention, B=4, S=1024, K=512,
H=1024, NH=16, HD=64, fp32.

Sharding: 8 cores = (batch b in 0..3) x (head-group hg in 0..1, 8 heads each).
All six projections are column-split by head-group; per-head attention is
entirely core-local; output columns are disjoint per core, so the gather is a
pure concatenation (no collectives).

Per-core pipeline (matmul operands in f32r = TF32-like single-pass PE mode,
~1.5e-4 rel err, full PE rate at free-dim >= 256):
  1. PE-transpose hs -> hsT [H, S] woven with the V projection; ehs -> ehsT
     transposed inside the first attention fill slot.
  2. Projections: QT/KT/KQT = W.T @ hsT (+bias), KKT = Wkk.T @ ehsT
     (transposed outputs); Vaug/KVaug = hs @ Wv in normal orientation with an
     augmented column of 2.0 (the ctxT matmul then also produces
     2*softmax-denominator, folding the (ctx+kctx)*0.5 average into the
     normalization for free).
  3. Per head h: scoresT[t,s] = K_h @ Q_h^T via lhsT=KT_h, rhs=QT_h
     (contraction HD=64); exp on ACT with per-partition mask bias and 1/8
     scale; ctxT_aug[65,S] += Vaug_h^T @ E accumulated over key chunks;
     PE-transpose back to [S, 64|den]; normalize + merge both branches on DVE.
  Remaining projections are interleaved between attention branches so the
  PE-heavy projection work fills the ACT-bound (exp) gaps; knowledge branches
  run delayed by 2 heads; output is DMA'd in two head-halves.
"""
import numpy as np
from contextlib import ExitStack

import concourse.bacc as bacc
import concourse.tile as tile
import concourse.mybir as mybir
from concourse.bass_utils import run_bass_kernel_spmd

F32 = mybir.dt.float32
F32R = mybir.dt.float32r
AF = mybir.ActivationFunctionType
ALU = mybir.AluOpType

P = 128
S = 1024        # query length
TKS = 1024      # self-branch key length
TKK = 512       # knowledge-branch key length
H = 1024        # model dim (projection contraction)
HG = 512        # per-core output width (8 heads x 64)
NHL = 8         # heads per core
HD = 64
HC = H // P     # 8 contraction chunks
INV = 0.125     # 1/sqrt(64)

_CACHE = {}
_DUMP = False


def _build():
    nc = bacc.Bacc(target_bir_lowering=False, debug=False)

    hs = nc.dram_tensor("hs", [S, H], F32, kind="ExternalInput")
    ehs = nc.dram_tensor("ehs", [TKK, H], F32, kind="ExternalInput")
    w_in = {}
    b_in = {}
    for nm in ["q", "k", "v", "kq", "kk", "kv"]:
        w_in[nm] = nc.dram_tensor(f"w{nm}", [H, HG], F32R, kind="ExternalInput")
        b_in[nm] = nc.dram_tensor(f"b{nm}", [HG], F32, kind="ExternalInput")
    mask = nc.dram_tensor("mask", [TKS], F32, kind="ExternalInput")
    emask = nc.dram_tensor("emask", [TKK], F32, kind="ExternalInput")
    out = nc.dram_tensor("out", [S, HG], F32, kind="ExternalOutput")

    with tile.TileContext(nc) as tc, ExitStack() as ctx:
        const = ctx.enter_context(tc.tile_pool(name="const", bufs=1))
        persist = ctx.enter_context(tc.tile_pool(name="persist", bufs=1))
        stage = ctx.enter_context(tc.tile_pool(name="stage", bufs=1))
        wpool = ctx.enter_context(tc.tile_pool(name="wpool", bufs=2))
        wvpool = ctx.enter_context(tc.tile_pool(name="wvpool", bufs=8))
        attp = ctx.enter_context(tc.tile_pool(name="att", bufs=3))
        ctxp = ctx.enter_context(tc.tile_pool(name="ctxp", bufs=2))
        psproj = ctx.enter_context(tc.tile_pool(name="psproj", bufs=2, space="PSUM"))

        # ---- constants ----
        ident_dram = nc.inline_tensor(np.eye(P, dtype=np.float32), name="ident_c")
        ident = const.tile([P, P], F32)
        nc.sync.dma_start(out=ident, in_=ident_dram.ap())
        mask_sb = const.tile([P, TKS // P], F32)
        nc.gpsimd.dma_start(out=mask_sb, in_=mask.ap().rearrange("(kt p) -> p kt", p=P))
        emask_sb = const.tile([P, TKK // P], F32)
        nc.gpsimd.dma_start(out=emask_sb, in_=emask.ap().rearrange("(kt p) -> p kt", p=P))
        bias_col = {}
        for nm in ["q", "k", "kq", "kk"]:
            t = const.tile([P, 4], F32, name=f"bias_{nm}")
            nc.gpsimd.dma_start(out=t, in_=b_in[nm].ap().rearrange("(jt p) -> p jt", p=P))
            bias_col[nm] = t
        bias_row = {}
        for nm in ["v", "kv"]:
            t = const.tile([P, HG], F32, name=f"brow_{nm}")
            nc.gpsimd.dma_start(out=t, in_=b_in[nm].ap().unsqueeze(0).broadcast_to([P, HG]))
            bias_row[nm] = t
        twos = const.tile([P, 1], F32)
        nc.vector.memset(twos, 2.0)

        # ---- persistent activations ----
        QT = persist.tile([P, 4, S], F32R)       # [j%128, jt, s]
        KT = persist.tile([P, 4, TKS], F32R)
        KQT = persist.tile([P, 4, S], F32R)
        KKT = persist.tile([P, 4, TKK], F32R)
        Vaug = persist.tile([P, TKS // P, NHL, HD + 1], F32R)   # [t%128, tt, h, d|2]
        KVaug = persist.tile([P, TKK // P, NHL, HD + 1], F32R)
        # output staging in two head-halves so the first DMA overlaps the tail
        out_half = [persist.tile([P, S // P, 4, HD], F32, name=f"out_half{i}",
                                 tag=f"out_half{i}") for i in range(2)]

        hsT = stage.tile([P, HC, S], F32R)       # [h%128, hc, s]
        ehsT = stage.tile([P, HC, TKK], F32R)

        # ---- stage-0 emitter: transpose one 128-row tile of hs/ehs ----
        def tp_tile(src, dstT, st, tag, bufs):
            h_tile = attp.tile([P, H], F32, name=f"h_stage_{tag}", tag=tag,
                               bufs=(3 if tag == "E" else bufs))
            # split the staging load so the first transposes start at half-load
            nc.sync.dma_start(out=h_tile[:, 0:H // 2],
                              in_=src[st * P:(st + 1) * P, 0:H // 2])
            nc.sync.dma_start(out=h_tile[:, H // 2:H],
                              in_=src[st * P:(st + 1) * P, H // 2:H])
            for hc0 in range(0, HC, 4):
                tp = psproj.tile([P, 4, P], F32, name="tp0", tag="psj")
                for i in range(4):
                    nc.tensor.transpose(
                        tp[:, i, :],
                        h_tile[:, (hc0 + i) * P:(hc0 + i + 1) * P], ident)
                nc.vector.tensor_copy(
                    dstT[:, hc0:hc0 + 4, st * P:(st + 1) * P], tp)

        # ---- projection emitters ----
        def proj_t_chunk(nm, dst, srcT, skeys, jt):
            wjt = wpool.tile([P, HC, P], F32R, name=f"w_{nm}_{jt}", tag="w")
            nc.sync.dma_start(
                out=wjt,
                in_=w_in[nm][:, jt * P:(jt + 1) * P].rearrange(
                    "(hc p) j -> p hc j", p=P))
            for sc in range(skeys // 512):
                ps = psproj.tile([P, 512], F32, name="psj", tag="psj")
                for hc in range(HC):
                    nc.tensor.matmul(
                        ps, lhsT=wjt[:, hc, :],
                        rhs=srcT[:, hc, sc * 512:(sc + 1) * 512],
                        start=(hc == 0), stop=(hc == HC - 1))
                nc.vector.tensor_scalar_add(
                    dst[:, jt, sc * 512:(sc + 1) * 512], ps,
                    bias_col[nm][:, jt:jt + 1])

        def proj_v_load(nm):
            wvs = []
            for hc in range(HC):
                wv = wvpool.tile([P, 512], F32R, name=f"wv_{nm}_{hc}", tag="wv")
                nc.sync.dma_start(out=wv, in_=w_in[nm][hc * P:(hc + 1) * P, :])
                wvs.append(wv)
            return wvs

        def proj_v_chunk(nm, dst, srcT, wvs, tt):
            ps = psproj.tile([P, 512], F32, name=f"psv{tt}", tag="psj")
            for hc in range(HC):
                nc.tensor.matmul(
                    ps, lhsT=srcT[:, hc, tt * P:(tt + 1) * P],
                    rhs=wvs[hc], start=(hc == 0), stop=(hc == HC - 1))
            nc.vector.scalar_tensor_tensor(
                out=dst[:, tt, :, 0:HD],
                in0=ps.rearrange("p (h d) -> p h d", h=NHL),
                scalar=1.0,
                in1=bias_row[nm].rearrange("p (h d) -> p h d", h=NHL),
                op0=ALU.mult, op1=ALU.add)
            nc.vector.tensor_copy(
                dst[:, tt, :, HD:HD + 1],
                twos.unsqueeze(1).broadcast_to([P, NHL, 1]))

        # ---- hs transposes woven with the V projection, then jt0 of Q/K ----
        wvs_v = None
        for st in range(S // P):
            tp_tile(hs, hsT, st, "E", 2)
            if wvs_v is None:
                wvs_v = proj_v_load("v")
            proj_v_chunk("v", Vaug, hsT, wvs_v, st)
        proj_t_chunk("q", QT, hsT, S, 0)
        proj_t_chunk("k", KT, hsT, TKS, 0)

        # ---- attention with interleaved remaining projections ----
        psbig = ctx.enter_context(tc.tile_pool(name="psbig", bufs=3, space="PSUM"))

        def head_branch(h, kt_mat, q_mat, vaug, n_keys, msk):
            base = (h % 2) * HD
            jt = h // 2
            ctx_ps = psbig.tile([65, S], F32, name=f"ctx_{h}_{n_keys}", tag="big")
            nkt = n_keys // P

            def ctx_mms(kt, e_sb):
                for sc in range(S // 512):
                    nc.tensor.matmul(
                        ctx_ps[:, sc * 512:(sc + 1) * 512],
                        lhsT=vaug[:, kt, h, :],
                        rhs=e_sb[:, sc * 512:(sc + 1) * 512],
                        start=(kt == 0), stop=(kt == nkt - 1))

            # software-pipelined: ctx(kt-1) is emitted after scores(kt), so the
            # PE overlaps exp(kt-1) latency with the next tile's score matmuls
            pend = []
            for kt in range(nkt):
                st_ps = psbig.tile([P, S], F32, name=f"st_{h}_{kt}", tag="big")
                for sc in range(S // 512):
                    nc.tensor.matmul(
                        st_ps[:, sc * 512:(sc + 1) * 512],
                        lhsT=kt_mat[base:base + HD, jt, kt * P:(kt + 1) * P],
                        rhs=q_mat[base:base + HD, jt, sc * 512:(sc + 1) * 512],
                        start=True, stop=True)
                e_sb = attp.tile([P, S], F32R, name=f"e_{h}_{kt}", tag="E", bufs=3)
                nc.scalar.activation(e_sb, st_ps, AF.Exp,
                                     bias=msk[:, kt:kt + 1], scale=INV)
                if _DUMP and h == 0 and kt == 0:
                    dtag = "s" if n_keys == TKS else "k"
                    d = nc.dram_tensor(f"d_e0{dtag}", [P, S], F32,
                                       kind="ExternalOutput")
                    nc.sync.dma_start(out=d.ap(), in_=e_sb.bitcast(F32))
                pend.append((kt, e_sb))
                if len(pend) > 2:
                    ctx_mms(*pend.pop(0))
            for p in pend:
                ctx_mms(*p)
            ctx_sb = ctxp.tile([65, S], F32, name=f"ctxsb_{h}_{n_keys}",
                               tag="ctx_sb", bufs=4)
            nc.vector.tensor_copy(ctx_sb, ctx_ps)
            if _DUMP and h == 0:
                dtag = "s" if n_keys == TKS else "k"
                d = nc.dram_tensor(f"d_ctx{dtag}", [65, S], F32,
                                   kind="ExternalOutput")
                nc.sync.dma_start(out=d.ap(), in_=ctx_sb)
            return ctx_sb

        def post(h, ctx_self, ctx_knl):
            oh = out_half[h // 4]
            for half in range(2):   # s-chunk groups 0-3 / 4-7
                tA = psbig.tile([P, 4, HD + 1], F32, name=f"tA_{h}_{half}",
                                tag="big")
                tB = psbig.tile([P, 4, HD + 1], F32, name=f"tB_{h}_{half}",
                                tag="big")
                for i in range(4):
                    sc = half * 4 + i
                    ssl = slice(sc * P, (sc + 1) * P)
                    nc.tensor.transpose(tA[:, i, :], ctx_self[:, ssl],
                                        ident[0:HD + 1, 0:HD + 1])
                    nc.tensor.transpose(tB[:, i, :], ctx_knl[:, ssl],
                                        ident[0:HD + 1, 0:HD + 1])
                rbs = ctxp.tile([P, 4, 1], F32, name=f"rbs_{h}_{half}", tag="rbs")
                rbk = ctxp.tile([P, 4, 1], F32, name=f"rbk_{h}_{half}", tag="rbk")
                nc.vector.reciprocal(rbs, tA[:, :, HD:HD + 1])
                nc.vector.reciprocal(rbk, tB[:, :, HD:HD + 1])
                msf = ctxp.tile([P, 4, HD], F32, name=f"msf_{h}_{half}", tag="msf", bufs=1)
                mkn = ctxp.tile([P, 4, HD], F32, name=f"mkn_{h}_{half}", tag="mkn", bufs=1)
                nc.vector.tensor_tensor(
                    out=msf, in0=tA[:, :, 0:HD],
                    in1=rbs.broadcast_to([P, 4, HD]), op=ALU.mult)
                nc.vector.tensor_tensor(
                    out=mkn, in0=tB[:, :, 0:HD],
                    in1=rbk.broadcast_to([P, 4, HD]), op=ALU.mult)
                nc.vector.tensor_tensor(
                    out=oh[:, half * 4:half * 4 + 4, h % 4, :],
                    in0=msf, in1=mkn, op=ALU.add)

        def self_branch(h):
            return head_branch(h, KT, QT, Vaug, TKS, mask_sb)

        def knl_branch(h):
            return head_branch(h, KKT, KQT, KVaug, TKK, emask_sb)

        def out_dma(i):
            nc.sync.dma_start(
                out=out.ap()[:, i * 256:(i + 1) * 256].rearrange(
                    "(sc p) j -> p sc j", p=P),
                in_=out_half[i].rearrange("p sc h d -> p sc (h d)"))

        ctx_self = {}
        ctx_knl = {}
        wvs_kv = None

        def fill(h):
            nonlocal wvs_kv
            if h == 0:
                for st in range(TKK // P):
                    tp_tile(ehs, ehsT, st, "ehst", 1)
                wvs_kv = proj_v_load("kv")
            elif h == 1:
                for tt in range(TKK // P):
                    proj_v_chunk("kv", KVaug, ehsT, wvs_kv, tt)
                proj_t_chunk("q", QT, hsT, S, 1)
                proj_t_chunk("k", KT, hsT, TKS, 1)
            elif h == 2:
                proj_t_chunk("kq", KQT, hsT, S, 0)
                proj_t_chunk("kk", KKT, ehsT, TKK, 0)
            elif h == 3:
                proj_t_chunk("q", QT, hsT, S, 2)
                proj_t_chunk("k", KT, hsT, TKS, 2)
                proj_t_chunk("kq", KQT, hsT, S, 1)
            elif h == 4:
                proj_t_chunk("kk", KKT, ehsT, TKK, 1)
                proj_t_chunk("q", QT, hsT, S, 3)
                proj_t_chunk("k", KT, hsT, TKS, 3)
            elif h == 5:
                proj_t_chunk("kq", KQT, hsT, S, 2)
                proj_t_chunk("kk", KKT, ehsT, TKK, 2)
            elif h == 6:
                proj_t_chunk("kq", KQT, hsT, S, 3)
                proj_t_chunk("kk", KKT, ehsT, TKK, 3)

        for h in range(NHL):
            ctx_self[h] = self_branch(h)
            fill(h)
            if h >= 2:
                hp = h - 2
                ctx_knl[hp] = knl_branch(hp)
                post(hp, ctx_self.pop(hp), ctx_knl.pop(hp))
                if hp == 3:
                    out_dma(0)
        for hp in range(NHL - 2, NHL):
            ctx_knl[hp] = knl_branch(hp)
            post(hp, ctx_self.pop(hp), ctx_knl.pop(hp))
        out_dma(1)

        if _DUMP:
            for nm, t in [("d_QT", QT), ("d_KT", KT),
                          ("d_KQT", KQT), ("d_KKT", KKT), ("d_Vaug", Vaug),
                          ("d_KVaug", KVaug), ("d_hsT", hsT)]:
                d = nc.dram_tensor(nm, list(t.shape), F32, kind="ExternalOutput")
                nc.sync.dma_start(out=d.ap(), in_=t.bitcast(F32) if t.dtype == F32R else t)

    nc.finalize()
    return nc


def _get_nc():
    if "nc" not in _CACHE:
        _CACHE["nc"] = _build()
    return _CACHE["nc"]


def kernel(**inputs):
    inp = {k: np.asarray(v, dtype=np.float32) for k, v in inputs.items()}
    nc = _get_nc()

    B = 4
    in_maps = []
    for core in range(8):
        b, hg = core // 2, core % 2
        sl = slice(hg * HG, (hg + 1) * HG)
        m = {
            "hs": np.ascontiguousarray(inp["hidden_states"][b]),
            "ehs": np.ascontiguousarray(inp["encoder_hidden_states"][b]),
            "mask": np.ascontiguousarray(inp["attention_mask"][b, 0, 0, :]),
            "emask": np.ascontiguousarray(inp["encoder_attention_mask"][b, 0, 0, :]),
        }
        for nm in ["q", "k", "v", "kq", "kk", "kv"]:
            m[f"w{nm}"] = np.ascontiguousarray(inp[f"W{nm}"][:, sl])
            m[f"b{nm}"] = np.ascontiguousarray(inp[f"b{nm}"][sl])
        in_maps.append(m)

    res = run_bass_kernel_spmd(nc, in_maps, core_ids=list(range(8)))

    outp = np.empty((B, S, H), np.float32)
    for core in range(8):
        b, hg = core // 2, core % 2
        outp[b, :, hg * HG:(hg + 1) * HG] = res.results[core]["out"]
    return outp


# revision 36
# speedup vs baseline: 1.0867x; 1.0046x over previous
"""Trainium2 Bass kernel for nn_BertSelfAttention_7962869367489.

Dual-branch (self + cross/"knowledge") BERT attention, B=4, S=1024, K=512,
H=1024, NH=16, HD=64, fp32.

Sharding: 8 cores = (batch b in 0..3) x (head-group hg in 0..1, 8 heads each).
All six projections are column-split by head-group; per-head attention is
entirely core-local; output columns are disjoint per core, so the gather is a
pure concatenation (no collectives).

Per-core pipeline (matmul operands in f32r = TF32-like single-pass PE mode,
~1.5e-4 rel err, full PE rate at free-dim >= 256):
  1. PE-transpose hs -> hsT [H, S] woven with the V projection; ehs -> ehsT
     transposed inside the first attention fill slot.
  2. Projections: QT/KT/KQT = W.T @ hsT (+bias), KKT = Wkk.T @ ehsT
     (transposed outputs); Vaug/KVaug = hs @ Wv in normal orientation with an
     augmented column of 2.0 (the ctxT matmul then also produces
     2*softmax-denominator, folding the (ctx+kctx)*0.5 average into the
     normalization for free).
  3. Per head h: scoresT[t,s] = K_h @ Q_h^T via lhsT=KT_h, rhs=QT_h
     (contraction HD=64); exp on ACT with per-partition mask bias and 1/8
     scale; ctxT_aug[65,S] += Vaug_h^T @ E accumulated over key chunks;
     PE-transpose back to [S, 64|den]; normalize + merge both branches on DVE.
  Remaining projections are interleaved between attention branches so the
  PE-heavy projection work fills the ACT-bound (exp) gaps; knowledge branches
  run delayed by 2 heads; output is DMA'd in two head-halves.
"""
import numpy as np
from contextlib import ExitStack

import concourse.bacc as bacc
import concourse.tile as tile
import concourse.mybir as mybir
from concourse.bass_utils import run_bass_kernel_spmd

F32 = mybir.dt.float32
F32R = mybir.dt.float32r
AF = mybir.ActivationFunctionType
ALU = mybir.AluOpType

P = 128
S = 1024        # query length
TKS = 1024      # self-branch key length
TKK = 512       # knowledge-branch key length
H = 1024        # model dim (projection contraction)
HG = 512        # per-core output width (8 heads x 64)
NHL = 8         # heads per core
HD = 64
HC = H // P     # 8 contraction chunks
INV = 0.125     # 1/sqrt(64)

_CACHE = {}
_DUMP = False


def _build():
    nc = bacc.Bacc(target_bir_lowering=False, debug=False)

    hs = nc.dram_tensor("hs", [S, H], F32, kind="ExternalInput")
    ehs = nc.dram_tensor("ehs", [TKK, H], F32, kind="ExternalInput")
    w_in = {}
    b_in = {}
    for nm in ["q", "k", "v", "kq", "kk", "kv"]:
        w_in[nm] = nc.dram_tensor(f"w{nm}", [H, HG], F32R, kind="ExternalInput")
        b_in[nm] = nc.dram_tensor(f"b{nm}", [HG], F32, kind="ExternalInput")
    mask = nc.dram_tensor("mask", [TKS], F32, kind="ExternalInput")
    emask = nc.dram_tensor("emask", [TKK], F32, kind="ExternalInput")
    out = nc.dram_tensor("out", [S, HG], F32, kind="ExternalOutput")

    with tile.TileContext(nc) as tc, ExitStack() as ctx:
        const = ctx.enter_context(tc.tile_pool(name="const", bufs=1))
        persist = ctx.enter_context(tc.tile_pool(name="persist", bufs=1))
        stage = ctx.enter_context(tc.tile_pool(name="stage", bufs=1))
        wpool = ctx.enter_context(tc.tile_pool(name="wpool", bufs=2))
        wvpool = ctx.enter_context(tc.tile_pool(name="wvpool", bufs=8))
        attp = ctx.enter_context(tc.tile_pool(name="att", bufs=3))
        ctxp = ctx.enter_context(tc.tile_pool(name="ctxp", bufs=2))
        psproj = ctx.enter_context(tc.tile_pool(name="psproj", bufs=2, space="PSUM"))

        # ---- constants ----
        ident_dram = nc.inline_tensor(np.eye(P, dtype=np.float32), name="ident_c")
        ident = const.tile([P, P], F32)
        nc.sync.dma_start(out=ident, in_=ident_dram.ap())
        mask_sb = const.tile([P, TKS // P], F32)
        nc.gpsimd.dma_start(out=mask_sb, in_=mask.ap().rearrange("(kt p) -> p kt", p=P))
        emask_sb = const.tile([P, TKK // P], F32)
        nc.gpsimd.dma_start(out=emask_sb, in_=emask.ap().rearrange("(kt p) -> p kt", p=P))
        bias_col = {}
        for nm in ["q", "k", "kq", "kk"]:
            t = const.tile([P, 4], F32, name=f"bias_{nm}")
            nc.gpsimd.dma_start(out=t, in_=b_in[nm].ap().rearrange("(jt p) -> p jt", p=P))
            bias_col[nm] = t
        bias_row = {}
        for nm in ["v", "kv"]:
            t = const.tile([P, HG], F32, name=f"brow_{nm}")
            nc.gpsimd.dma_start(out=t, in_=b_in[nm].ap().unsqueeze(0).broadcast_to([P, HG]))
            bias_row[nm] = t
        twos = const.tile([P, 1], F32)
        nc.vector.memset(twos, 2.0)

        # ---- persistent activations ----
        QT = persist.tile([P, 4, S], F32R)       # [j%128, jt, s]
        KT = persist.tile([P, 4, TKS], F32R)
        KQT = persist.tile([P, 4, S], F32R)
        KKT = persist.tile([P, 4, TKK], F32R)
        Vaug = persist.tile([P, TKS // P, NHL, HD + 1], F32R)   # [t%128, tt, h, d|2]
        KVaug = persist.tile([P, TKK // P, NHL, HD + 1], F32R)
        # output staging in two head-halves so the first DMA overlaps the tail
        out_half = [persist.tile([P, S // P, 4, HD], F32, name=f"out_half{i}",
                                 tag=f"out_half{i}") for i in range(2)]

        hsT = stage.tile([P, HC, S], F32R)       # [h%128, hc, s]
        ehsT = stage.tile([P, HC, TKK], F32R)

        # ---- stage-0 emitter: transpose one 128-row tile of hs/ehs ----
        def tp_tile(src, dstT, st, tag, bufs):
            h_tile = attp.tile([P, H], F32, name=f"h_stage_{tag}", tag=tag,
                               bufs=(3 if tag == "E" else bufs))
            # split the staging load so the first transposes start at half-load
            nc.sync.dma_start(out=h_tile[:, 0:H // 2],
                              in_=src[st * P:(st + 1) * P, 0:H // 2])
            nc.sync.dma_start(out=h_tile[:, H // 2:H],
                              in_=src[st * P:(st + 1) * P, H // 2:H])
            for hc0 in range(0, HC, 4):
                tp = psproj.tile([P, 4, P], F32, name="tp0", tag="psj")
                for i in range(4):
                    nc.tensor.transpose(
                        tp[:, i, :],
                        h_tile[:, (hc0 + i) * P:(hc0 + i + 1) * P], ident)
                nc.vector.tensor_copy(
                    dstT[:, hc0:hc0 + 4, st * P:(st + 1) * P], tp)

        # ---- projection emitters ----
        def proj_t_chunk(nm, dst, srcT, skeys, jt):
            wjt = wpool.tile([P, HC, P], F32R, name=f"w_{nm}_{jt}", tag="w")
            nc.sync.dma_start(
                out=wjt,
                in_=w_in[nm][:, jt * P:(jt + 1) * P].rearrange(
                    "(hc p) j -> p hc j", p=P))
            for sc in range(skeys // 512):
                ps = psproj.tile([P, 512], F32, name="psj", tag="psj")
                for hc in range(HC):
                    nc.tensor.matmul(
                        ps, lhsT=wjt[:, hc, :],
                        rhs=srcT[:, hc, sc * 512:(sc + 1) * 512],
                        start=(hc == 0), stop=(hc == HC - 1))
                nc.vector.tensor_scalar_add(
                    dst[:, jt, sc * 512:(sc + 1) * 512], ps,
                    bias_col[nm][:, jt:jt + 1])

        def proj_v_load(nm):
            wvs = []
            for hc in range(HC):
                wv = wvpool.tile([P, 512], F32R, name=f"wv_{nm}_{hc}", tag="wv")
                nc.sync.dma_start(out=wv, in_=w_in[nm][hc * P:(hc + 1) * P, :])
                wvs.append(wv)
            return wvs

        def proj_v_chunk(nm, dst, srcT, wvs, tt):
            ps = psproj.tile([P, 512], F32, name=f"psv{tt}", tag="psj")
            for hc in range(HC):
                nc.tensor.matmul(
                    ps, lhsT=srcT[:, hc, tt * P:(tt + 1) * P],
                    rhs=wvs[hc], start=(hc == 0), stop=(hc == HC - 1))
            nc.vector.scalar_tensor_tensor(
                out=dst[:, tt, :, 0:HD],
                in0=ps.rearrange("p (h d) -> p h d", h=NHL),
                scalar=1.0,
                in1=bias_row[nm].rearrange("p (h d) -> p h d", h=NHL),
                op0=ALU.mult, op1=ALU.add)
            nc.vector.tensor_copy(
                dst[:, tt, :, HD:HD + 1],
                twos.unsqueeze(1).broadcast_to([P, NHL, 1]))

        # ---- hs transposes woven with the V projection, then jt0 of Q/K ----
        wvs_v = None
        for st in range(S // P):
            tp_tile(hs, hsT, st, "E", 2)
            if wvs_v is None:
                wvs_v = proj_v_load("v")
            proj_v_chunk("v", Vaug, hsT, wvs_v, st)
        proj_t_chunk("q", QT, hsT, S, 0)
        proj_t_chunk("k", KT, hsT, TKS, 0)

        # ---- attention with interleaved remaining projections ----
        psbig = ctx.enter_context(tc.tile_pool(name="psbig", bufs=3, space="PSUM"))

        def head_branch(h, kt_mat, q_mat, vaug, n_keys, msk):
            base = (h % 2) * HD
            jt = h // 2
            ctx_ps = psbig.tile([65, S], F32, name=f"ctx_{h}_{n_keys}", tag="big")
            nkt = n_keys // P

            def ctx_mms(kt, e_sb):
                for sc in range(S // 512):
                    nc.tensor.matmul(
                        ctx_ps[:, sc * 512:(sc + 1) * 512],
                        lhsT=vaug[:, kt, h, :],
                        rhs=e_sb[:, sc * 512:(sc + 1) * 512],
                        start=(kt == 0), stop=(kt == nkt - 1))

            # software-pipelined: ctx(kt-1) is emitted after scores(kt), so the
            # PE overlaps exp(kt-1) latency with the next tile's score matmuls
            pend = []
            for kt in range(nkt):
                st_ps = psbig.tile([P, S], F32, name=f"st_{h}_{kt}", tag="big")
                for sc in range(S // 512):
                    nc.tensor.matmul(
                        st_ps[:, sc * 512:(sc + 1) * 512],
                        lhsT=kt_mat[base:base + HD, jt, kt * P:(kt + 1) * P],
                        rhs=q_mat[base:base + HD, jt, sc * 512:(sc + 1) * 512],
                        start=True, stop=True)
                e_sb = attp.tile([P, S], F32R, name=f"e_{h}_{kt}", tag="E", bufs=3)
                nc.scalar.activation(e_sb, st_ps, AF.Exp,
                                     bias=msk[:, kt:kt + 1], scale=INV)
                if _DUMP and h == 0 and kt == 0:
                    dtag = "s" if n_keys == TKS else "k"
                    d = nc.dram_tensor(f"d_e0{dtag}", [P, S], F32,
                                       kind="ExternalOutput")
                    nc.sync.dma_start(out=d.ap(), in_=e_sb.bitcast(F32))
                pend.append((kt, e_sb))
                if len(pend) > 2:
                    ctx_mms(*pend.pop(0))
            ctx_sb = ctxp.tile([65, S], F32, name=f"ctxsb_{h}_{n_keys}",
                               tag="ctx_sb", bufs=4)

            def flush():
                for p in pend:
                    ctx_mms(*p)
                nc.vector.tensor_copy(ctx_sb, ctx_ps)
                if _DUMP and h == 0:
                    dtag = "s" if n_keys == TKS else "k"
                    d = nc.dram_tensor(f"d_ctx{dtag}", [65, S], F32,
                                       kind="ExternalOutput")
                    nc.sync.dma_start(out=d.ap(), in_=ctx_sb)

            return ctx_sb, flush

        def post(h, ctx_self, ctx_knl):
            oh = out_half[h // 4]
            for half in range(2):   # s-chunk groups 0-3 / 4-7
                tA = psbig.tile([P, 4, HD + 1], F32, name=f"tA_{h}_{half}",
                                tag="big")
                tB = psbig.tile([P, 4, HD + 1], F32, name=f"tB_{h}_{half}",
                                tag="big")
                for i in range(4):
                    sc = half * 4 + i
                    ssl = slice(sc * P, (sc + 1) * P)
                    nc.tensor.transpose(tA[:, i, :], ctx_self[:, ssl],
                                        ident[0:HD + 1, 0:HD + 1])
                    nc.tensor.transpose(tB[:, i, :], ctx_knl[:, ssl],
                                        ident[0:HD + 1, 0:HD + 1])
                rbs = ctxp.tile([P, 4, 1], F32, name=f"rbs_{h}_{half}", tag="rbs")
                rbk = ctxp.tile([P, 4, 1], F32, name=f"rbk_{h}_{half}", tag="rbk")
                nc.vector.reciprocal(rbs, tA[:, :, HD:HD + 1])
                nc.vector.reciprocal(rbk, tB[:, :, HD:HD + 1])
                msf = ctxp.tile([P, 4, HD], F32, name=f"msf_{h}_{half}", tag="msf", bufs=1)
                mkn = ctxp.tile([P, 4, HD], F32, name=f"mkn_{h}_{half}", tag="mkn", bufs=1)
                nc.vector.tensor_tensor(
                    out=msf, in0=tA[:, :, 0:HD],
                    in1=rbs.broadcast_to([P, 4, HD]), op=ALU.mult)
                nc.vector.tensor_tensor(
                    out=mkn, in0=tB[:, :, 0:HD],
                    in1=rbk.broadcast_to([P, 4, HD]), op=ALU.mult)
                nc.vector.tensor_tensor(
                    out=oh[:, half * 4:half * 4 + 4, h % 4, :],
                    in0=msf, in1=mkn, op=ALU.add)

        def self_branch(h):
            return head_branch(h, KT, QT, Vaug, TKS, mask_sb)

        def knl_branch(h):
            return head_branch(h, KKT, KQT, KVaug, TKK, emask_sb)

        def out_dma(i):
            nc.sync.dma_start(
                out=out.ap()[:, i * 256:(i + 1) * 256].rearrange(
                    "(sc p) j -> p sc j", p=P),
                in_=out_half[i].rearrange("p sc h d -> p sc (h d)"))

        ctx_self = {}
        ctx_knl = {}
        wvs_kv = None

        def fill(h):
            nonlocal wvs_kv
            if h == 0:
                for st in range(TKK // P):
                    tp_tile(ehs, ehsT, st, "ehst", 1)
                wvs_kv = proj_v_load("kv")
            elif h == 1:
                for tt in range(TKK // P):
                    proj_v_chunk("kv", KVaug, ehsT, wvs_kv, tt)
                proj_t_chunk("q", QT, hsT, S, 1)
                proj_t_chunk("k", KT, hsT, TKS, 1)
            elif h == 2:
                proj_t_chunk("kq", KQT, hsT, S, 0)
                proj_t_chunk("kk", KKT, ehsT, TKK, 0)
            elif h == 3:
                proj_t_chunk("q", QT, hsT, S, 2)
                proj_t_chunk("k", KT, hsT, TKS, 2)
                proj_t_chunk("kq", KQT, hsT, S, 1)
            elif h == 4:
                proj_t_chunk("kk", KKT, ehsT, TKK, 1)
                proj_t_chunk("q", QT, hsT, S, 3)
                proj_t_chunk("k", KT, hsT, TKS, 3)
            elif h == 5:
                proj_t_chunk("kq", KQT, hsT, S, 2)
                proj_t_chunk("kk", KKT, ehsT, TKK, 2)
            elif h == 6:
                proj_t_chunk("kq", KQT, hsT, S, 3)
                proj_t_chunk("kk", KKT, ehsT, TKK, 3)

        for h in range(NHL):
            ctx_self[h], sflush = self_branch(h)
            fill(h)
            sflush()
            if h >= 2:
                hp = h - 2
                ctx_knl[hp], kflush = knl_branch(hp)
                kflush()
                post(hp, ctx_self.pop(hp), ctx_knl.pop(hp))
                if hp == 3:
                    out_dma(0)
        for hp in range(NHL - 2, NHL):
            ctx_knl[hp], kflush = knl_branch(hp)
            kflush()
            post(hp, ctx_self.pop(hp), ctx_knl.pop(hp))
        out_dma(1)

        if _DUMP:
            for nm, t in [("d_QT", QT), ("d_KT", KT),
                          ("d_KQT", KQT), ("d_KKT", KKT), ("d_Vaug", Vaug),
                          ("d_KVaug", KVaug), ("d_hsT", hsT)]:
                d = nc.dram_tensor(nm, list(t.shape), F32, kind="ExternalOutput")
                nc.sync.dma_start(out=d.ap(), in_=t.bitcast(F32) if t.dtype == F32R else t)

    nc.finalize()
    return nc


def _get_nc():
    if "nc" not in _CACHE:
        _CACHE["nc"] = _build()
    return _CACHE["nc"]


def kernel(**inputs):
    inp = {k: np.asarray(v, dtype=np.float32) for k, v in inputs.items()}
    nc = _get_nc()

    B = 4
    in_maps = []
    for core in range(8):
        b, hg = core // 2, core % 2
        sl = slice(hg * HG, (hg + 1) * HG)
        m = {
            "hs": np.ascontiguousarray(inp["hidden_states"][b]),
            "ehs": np.ascontiguousarray(inp["encoder_hidden_states"][b]),
            "mask": np.ascontiguousarray(inp["attention_mask"][b, 0, 0, :]),
            "emask": np.ascontiguousarray(inp["encoder_attention_mask"][b, 0, 0, :]),
        }
        for nm in ["q", "k", "v", "kq", "kk", "kv"]:
            m[f"w{nm}"] = np.ascontiguousarray(inp[f"W{nm}"][:, sl])
            m[f"b{nm}"] = np.ascontiguousarray(inp[f"b{nm}"][sl])
        in_maps.append(m)

    res = run_bass_kernel_spmd(nc, in_maps, core_ids=list(range(8)))

    outp = np.empty((B, S, H), np.float32)
    for core in range(8):
        b, hg = core // 2, core % 2
        outp[b, :, hg * HG:(hg + 1) * HG] = res.results[core]["out"]
    return outp


# revision 37
# speedup vs baseline: 1.0939x; 1.0066x over previous
"""Trainium2 Bass kernel for nn_BertSelfAttention_7962869367489.

Dual-branch (self + cross/"knowledge") BERT attention, B=4, S=1024, K=512,
H=1024, NH=16, HD=64, fp32.

Sharding: 8 cores = (batch b in 0..3) x (head-group hg in 0..1, 8 heads each).
All six projections are column-split by head-group; per-head attention is
entirely core-local; output columns are disjoint per core, so the gather is a
pure concatenation (no collectives).

Per-core pipeline (matmul operands in f32r = TF32-like single-pass PE mode,
~1.5e-4 rel err, full PE rate at free-dim >= 256):
  1. PE-transpose hs -> hsT [H, S] woven with the V projection; ehs -> ehsT
     transposed inside the first attention fill slot.
  2. Projections: QT/KT/KQT = W.T @ hsT (+bias), KKT = Wkk.T @ ehsT
     (transposed outputs); Vaug/KVaug = hs @ Wv in normal orientation with an
     augmented column of 2.0 (the ctxT matmul then also produces
     2*softmax-denominator, folding the (ctx+kctx)*0.5 average into the
     normalization for free).
  3. Per head h: scoresT[t,s] = K_h @ Q_h^T via lhsT=KT_h, rhs=QT_h
     (contraction HD=64); exp on ACT with per-partition mask bias and 1/8
     scale; ctxT_aug[65,S] += Vaug_h^T @ E accumulated over key chunks;
     PE-transpose back to [S, 64|den]; normalize + merge both branches on DVE.
  Remaining projections are interleaved between attention branches so the
  PE-heavy projection work fills the ACT-bound (exp) gaps; knowledge branches
  run delayed by 2 heads; output is DMA'd in two head-halves.
"""
import numpy as np
from contextlib import ExitStack

import concourse.bacc as bacc
import concourse.tile as tile
import concourse.mybir as mybir
from concourse.bass_utils import run_bass_kernel_spmd

F32 = mybir.dt.float32
F32R = mybir.dt.float32r
AF = mybir.ActivationFunctionType
ALU = mybir.AluOpType

P = 128
S = 1024        # query length
TKS = 1024      # self-branch key length
TKK = 512       # knowledge-branch key length
H = 1024        # model dim (projection contraction)
HG = 512        # per-core output width (8 heads x 64)
NHL = 8         # heads per core
HD = 64
HC = H // P     # 8 contraction chunks
INV = 0.125     # 1/sqrt(64)

_CACHE = {}
_DUMP = False


def _build():
    nc = bacc.Bacc(target_bir_lowering=False, debug=False)

    hs = nc.dram_tensor("hs", [S, H], F32, kind="ExternalInput")
    ehs = nc.dram_tensor("ehs", [TKK, H], F32, kind="ExternalInput")
    w_in = {}
    b_in = {}
    for nm in ["q", "k", "v", "kq", "kk", "kv"]:
        w_in[nm] = nc.dram_tensor(f"w{nm}", [H, HG], F32R, kind="ExternalInput")
        b_in[nm] = nc.dram_tensor(f"b{nm}", [HG], F32, kind="ExternalInput")
    mask = nc.dram_tensor("mask", [TKS], F32, kind="ExternalInput")
    emask = nc.dram_tensor("emask", [TKK], F32, kind="ExternalInput")
    out = nc.dram_tensor("out", [S, HG], F32, kind="ExternalOutput")

    with tile.TileContext(nc) as tc, ExitStack() as ctx:
        const = ctx.enter_context(tc.tile_pool(name="const", bufs=1))
        persist = ctx.enter_context(tc.tile_pool(name="persist", bufs=1))
        stage = ctx.enter_context(tc.tile_pool(name="stage", bufs=1))
        wpool = ctx.enter_context(tc.tile_pool(name="wpool", bufs=2))
        wvpool = ctx.enter_context(tc.tile_pool(name="wvpool", bufs=8))
        attp = ctx.enter_context(tc.tile_pool(name="att", bufs=3))
        ctxp = ctx.enter_context(tc.tile_pool(name="ctxp", bufs=2))
        psproj = ctx.enter_context(tc.tile_pool(name="psproj", bufs=2, space="PSUM"))

        # ---- constants ----
        ident_dram = nc.inline_tensor(np.eye(P, dtype=np.float32), name="ident_c")
        ident = const.tile([P, P], F32)
        nc.sync.dma_start(out=ident, in_=ident_dram.ap())
        mask_sb = const.tile([P, TKS // P], F32)
        nc.gpsimd.dma_start(out=mask_sb, in_=mask.ap().rearrange("(kt p) -> p kt", p=P))
        emask_sb = const.tile([P, TKK // P], F32)
        nc.gpsimd.dma_start(out=emask_sb, in_=emask.ap().rearrange("(kt p) -> p kt", p=P))
        bias_col = {}
        for nm in ["q", "k", "kq", "kk"]:
            t = const.tile([P, 4], F32, name=f"bias_{nm}")
            nc.gpsimd.dma_start(out=t, in_=b_in[nm].ap().rearrange("(jt p) -> p jt", p=P))
            bias_col[nm] = t
        bias_row = {}
        for nm in ["v", "kv"]:
            t = const.tile([P, HG], F32, name=f"brow_{nm}")
            nc.gpsimd.dma_start(out=t, in_=b_in[nm].ap().unsqueeze(0).broadcast_to([P, HG]))
            bias_row[nm] = t
        twos = const.tile([P, 1], F32)
        nc.vector.memset(twos, 2.0)

        # ---- persistent activations ----
        QT = persist.tile([P, 4, S], F32R)       # [j%128, jt, s]
        KT = persist.tile([P, 4, TKS], F32R)
        KQT = persist.tile([P, 4, S], F32R)
        KKT = persist.tile([P, 4, TKK], F32R)
        Vaug = persist.tile([P, TKS // P, NHL, HD + 1], F32R)   # [t%128, tt, h, d|2]
        KVaug = persist.tile([P, TKK // P, NHL, HD + 1], F32R)
        # output staging in two head-halves so the first DMA overlaps the tail
        out_half = [persist.tile([P, S // P, 4, HD], F32, name=f"out_half{i}",
                                 tag=f"out_half{i}") for i in range(2)]

        hsT = stage.tile([P, HC, S], F32R)       # [h%128, hc, s]
        ehsT = stage.tile([P, HC, TKK], F32R)

        # ---- stage-0 emitter: transpose one 128-row tile of hs/ehs ----
        def tp_tile(src, dstT, st, tag, bufs):
            h_tile = attp.tile([P, H], F32, name=f"h_stage_{tag}", tag=tag,
                               bufs=(3 if tag == "E" else bufs))
            # split the staging load so the first transposes start at half-load
            nc.sync.dma_start(out=h_tile[:, 0:H // 2],
                              in_=src[st * P:(st + 1) * P, 0:H // 2])
            nc.sync.dma_start(out=h_tile[:, H // 2:H],
                              in_=src[st * P:(st + 1) * P, H // 2:H])
            for hc0 in range(0, HC, 4):
                tp = psproj.tile([P, 4, P], F32, name="tp0", tag="psj")
                for i in range(4):
                    nc.tensor.transpose(
                        tp[:, i, :],
                        h_tile[:, (hc0 + i) * P:(hc0 + i + 1) * P], ident)
                nc.vector.tensor_copy(
                    dstT[:, hc0:hc0 + 4, st * P:(st + 1) * P], tp)

        # ---- projection emitters ----
        def proj_t_chunk(nm, dst, srcT, skeys, jt):
            wjt = wpool.tile([P, HC, P], F32R, name=f"w_{nm}_{jt}", tag="w")
            nc.sync.dma_start(
                out=wjt,
                in_=w_in[nm][:, jt * P:(jt + 1) * P].rearrange(
                    "(hc p) j -> p hc j", p=P))
            for sc in range(skeys // 512):
                ps = psproj.tile([P, 512], F32, name="psj", tag="psj")
                for hc in range(HC):
                    nc.tensor.matmul(
                        ps, lhsT=wjt[:, hc, :],
                        rhs=srcT[:, hc, sc * 512:(sc + 1) * 512],
                        start=(hc == 0), stop=(hc == HC - 1))
                nc.vector.tensor_scalar_add(
                    dst[:, jt, sc * 512:(sc + 1) * 512], ps,
                    bias_col[nm][:, jt:jt + 1])

        def proj_v_load(nm):
            wvs = []
            for hc in range(HC):
                wv = wvpool.tile([P, 512], F32R, name=f"wv_{nm}_{hc}", tag="wv")
                nc.sync.dma_start(out=wv, in_=w_in[nm][hc * P:(hc + 1) * P, :])
                wvs.append(wv)
            return wvs

        def proj_v_chunk(nm, dst, srcT, wvs, tt):
            ps = psproj.tile([P, 512], F32, name=f"psv{tt}", tag="psj")
            for hc in range(HC):
                nc.tensor.matmul(
                    ps, lhsT=srcT[:, hc, tt * P:(tt + 1) * P],
                    rhs=wvs[hc], start=(hc == 0), stop=(hc == HC - 1))
            nc.vector.scalar_tensor_tensor(
                out=dst[:, tt, :, 0:HD],
                in0=ps.rearrange("p (h d) -> p h d", h=NHL),
                scalar=1.0,
                in1=bias_row[nm].rearrange("p (h d) -> p h d", h=NHL),
                op0=ALU.mult, op1=ALU.add)
            nc.vector.tensor_copy(
                dst[:, tt, :, HD:HD + 1],
                twos.unsqueeze(1).broadcast_to([P, NHL, 1]))

        # ---- hs transposes woven with the V projection, then jt0 of Q/K ----
        wvs_v = None
        for st in range(S // P):
            tp_tile(hs, hsT, st, "E", 2)
            if wvs_v is None:
                wvs_v = proj_v_load("v")
            proj_v_chunk("v", Vaug, hsT, wvs_v, st)
        proj_t_chunk("q", QT, hsT, S, 0)
        proj_t_chunk("k", KT, hsT, TKS, 0)

        # ---- attention with interleaved remaining projections ----
        psbig = ctx.enter_context(tc.tile_pool(name="psbig", bufs=3, space="PSUM"))

        def head_branch(h, kt_mat, q_mat, vaug, n_keys, msk):
            base = (h % 2) * HD
            jt = h // 2
            ctx_ps = psbig.tile([65, S], F32, name=f"ctx_{h}_{n_keys}", tag="big")
            nkt = n_keys // P

            def ctx_mms(kt, e_sb):
                for sc in range(S // 512):
                    nc.tensor.matmul(
                        ctx_ps[:, sc * 512:(sc + 1) * 512],
                        lhsT=vaug[:, kt, h, :],
                        rhs=e_sb[:, sc * 512:(sc + 1) * 512],
                        start=(kt == 0), stop=(kt == nkt - 1))

            # software-pipelined: ctx(kt-1) is emitted after scores(kt), so the
            # PE overlaps exp(kt-1) latency with the next tile's score matmuls
            pend = []
            for kt in range(nkt):
                st_ps = psbig.tile([P, S], F32, name=f"st_{h}_{kt}", tag="big")
                for sc in range(S // 512):
                    nc.tensor.matmul(
                        st_ps[:, sc * 512:(sc + 1) * 512],
                        lhsT=kt_mat[base:base + HD, jt, kt * P:(kt + 1) * P],
                        rhs=q_mat[base:base + HD, jt, sc * 512:(sc + 1) * 512],
                        start=True, stop=True)
                e_sb = attp.tile([P, S], F32R, name=f"e_{h}_{kt}", tag="E", bufs=3)
                nc.scalar.activation(e_sb, st_ps, AF.Exp,
                                     bias=msk[:, kt:kt + 1], scale=INV)
                if _DUMP and h == 0 and kt == 0:
                    dtag = "s" if n_keys == TKS else "k"
                    d = nc.dram_tensor(f"d_e0{dtag}", [P, S], F32,
                                       kind="ExternalOutput")
                    nc.sync.dma_start(out=d.ap(), in_=e_sb.bitcast(F32))
                pend.append((kt, e_sb))
                if len(pend) > 2:
                    ctx_mms(*pend.pop(0))
            ctx_sb = ctxp.tile([65, S], F32, name=f"ctxsb_{h}_{n_keys}",
                               tag="ctx_sb", bufs=4)

            def flush():
                for p in pend:
                    ctx_mms(*p)
                nc.vector.tensor_copy(ctx_sb, ctx_ps)
                if _DUMP and h == 0:
                    dtag = "s" if n_keys == TKS else "k"
                    d = nc.dram_tensor(f"d_ctx{dtag}", [65, S], F32,
                                       kind="ExternalOutput")
                    nc.sync.dma_start(out=d.ap(), in_=ctx_sb)

            return ctx_sb, flush

        def post(h, ctx_self, ctx_knl):
            oh = out_half[h // 4]
            for half in range(2):   # s-chunk groups 0-3 / 4-7
                tA = psbig.tile([P, 4, HD + 1], F32, name=f"tA_{h}_{half}",
                                tag="big")
                tB = psbig.tile([P, 4, HD + 1], F32, name=f"tB_{h}_{half}",
                                tag="big")
                for i in range(4):
                    sc = half * 4 + i
                    ssl = slice(sc * P, (sc + 1) * P)
                    nc.tensor.transpose(tA[:, i, :], ctx_self[:, ssl],
                                        ident[0:HD + 1, 0:HD + 1])
                    nc.tensor.transpose(tB[:, i, :], ctx_knl[:, ssl],
                                        ident[0:HD + 1, 0:HD + 1])
                rbs = ctxp.tile([P, 4, 1], F32, name=f"rbs_{h}_{half}", tag="rbs")
                rbk = ctxp.tile([P, 4, 1], F32, name=f"rbk_{h}_{half}", tag="rbk")
                nc.vector.reciprocal(rbs, tA[:, :, HD:HD + 1])
                nc.vector.reciprocal(rbk, tB[:, :, HD:HD + 1])
                msf = ctxp.tile([P, 4, HD], F32, name=f"msf_{h}_{half}", tag="msf", bufs=1)
                mkn = ctxp.tile([P, 4, HD], F32, name=f"mkn_{h}_{half}", tag="mkn", bufs=1)
                nc.vector.tensor_tensor(
                    out=msf, in0=tA[:, :, 0:HD],
                    in1=rbs.broadcast_to([P, 4, HD]), op=ALU.mult)
                nc.vector.tensor_tensor(
                    out=mkn, in0=tB[:, :, 0:HD],
                    in1=rbk.broadcast_to([P, 4, HD]), op=ALU.mult)
                nc.vector.tensor_tensor(
                    out=oh[:, half * 4:half * 4 + 4, h % 4, :],
                    in0=msf, in1=mkn, op=ALU.add)

        def self_branch(h):
            return head_branch(h, KT, QT, Vaug, TKS, mask_sb)

        def knl_branch(h):
            return head_branch(h, KKT, KQT, KVaug, TKK, emask_sb)

        def out_dma(i):
            nc.sync.dma_start(
                out=out.ap()[:, i * 256:(i + 1) * 256].rearrange(
                    "(sc p) j -> p sc j", p=P),
                in_=out_half[i].rearrange("p sc h d -> p sc (h d)"))

        ctx_self = {}
        ctx_knl = {}
        wvs_kv = None

        def fill(h):
            nonlocal wvs_kv
            if h == 0:
                for st in range(TKK // P):
                    tp_tile(ehs, ehsT, st, "ehst", 1)
                wvs_kv = proj_v_load("kv")
            elif h == 1:
                for tt in range(TKK // P):
                    proj_v_chunk("kv", KVaug, ehsT, wvs_kv, tt)
                proj_t_chunk("q", QT, hsT, S, 1)
                proj_t_chunk("k", KT, hsT, TKS, 1)
            elif h == 2:
                proj_t_chunk("kq", KQT, hsT, S, 0)
                proj_t_chunk("kk", KKT, ehsT, TKK, 0)
            elif h == 3:
                proj_t_chunk("q", QT, hsT, S, 2)
                proj_t_chunk("k", KT, hsT, TKS, 2)
                proj_t_chunk("kq", KQT, hsT, S, 1)
            elif h == 4:
                proj_t_chunk("kk", KKT, ehsT, TKK, 1)
                proj_t_chunk("q", QT, hsT, S, 3)
                proj_t_chunk("k", KT, hsT, TKS, 3)
            elif h == 5:
                proj_t_chunk("kq", KQT, hsT, S, 2)
                proj_t_chunk("kk", KKT, ehsT, TKK, 2)
            elif h == 6:
                proj_t_chunk("kq", KQT, hsT, S, 3)
                proj_t_chunk("kk", KKT, ehsT, TKK, 3)

        for h in range(NHL):
            ctx_self[h], sflush = self_branch(h)
            fill(h)
            sflush()
            if h >= 2:
                hp = h - 2
                ctx_knl[hp], kflush = knl_branch(hp)
                kflush()
                post(hp, ctx_self.pop(hp), ctx_knl.pop(hp))
                if hp == 3:
                    out_dma(0)
        ctx_knl[6], kf6 = knl_branch(6)
        ctx_knl[7], kf7 = knl_branch(7)
        kf6()
        post(6, ctx_self.pop(6), ctx_knl.pop(6))
        kf7()
        post(7, ctx_self.pop(7), ctx_knl.pop(7))
        out_dma(1)

        if _DUMP:
            for nm, t in [("d_QT", QT), ("d_KT", KT),
                          ("d_KQT", KQT), ("d_KKT", KKT), ("d_Vaug", Vaug),
                          ("d_KVaug", KVaug), ("d_hsT", hsT)]:
                d = nc.dram_tensor(nm, list(t.shape), F32, kind="ExternalOutput")
                nc.sync.dma_start(out=d.ap(), in_=t.bitcast(F32) if t.dtype == F32R else t)

    nc.finalize()
    return nc


def _get_nc():
    if "nc" not in _CACHE:
        _CACHE["nc"] = _build()
    return _CACHE["nc"]


def kernel(**inputs):
    inp = {k: np.asarray(v, dtype=np.float32) for k, v in inputs.items()}
    nc = _get_nc()

    B = 4
    in_maps = []
    for core in range(8):
        b, hg = core // 2, core % 2
        sl = slice(hg * HG, (hg + 1) * HG)
        m = {
            "hs": np.ascontiguousarray(inp["hidden_states"][b]),
            "ehs": np.ascontiguousarray(inp["encoder_hidden_states"][b]),
            "mask": np.ascontiguousarray(inp["attention_mask"][b, 0, 0, :]),
            "emask": np.ascontiguousarray(inp["encoder_attention_mask"][b, 0, 0, :]),
        }
        for nm in ["q", "k", "v", "kq", "kk", "kv"]:
            m[f"w{nm}"] = np.ascontiguousarray(inp[f"W{nm}"][:, sl])
            m[f"b{nm}"] = np.ascontiguousarray(inp[f"b{nm}"][sl])
        in_maps.append(m)

    res = run_bass_kernel_spmd(nc, in_maps, core_ids=list(range(8)))

    outp = np.empty((B, S, H), np.float32)
    for core in range(8):
        b, hg = core // 2, core % 2
        outp[b, :, hg * HG:(hg + 1) * HG] = res.results[core]["out"]
    return outp
